# revision 75
# baseline (speedup 1.0000x reference)
"""Causal self-attention (B=2, T=2048, C=1024, H=16) on 8 trn2 NeuronCores.

Sharding: tensor-parallel over heads - 2 heads per core. Each core computes
its heads' qkv projection (column-split w_attn), causal attention, and a
row-split partial of the output projection; the host sums the 8 fp16
partials and adds the biases.

Design notes (driven by the TimelineSim cost model, which bills a matmul
as moving-rows x cycle regardless of K/M):
  - fp16 everywhere on the PE (1 cyc/row unconditionally; fp32r pays 4x
    below 256 moving rows; fp8 tested and rejected: 3e-2 rel err).
  - q,k produced transposed (qT/kT [128, T], moving data = x chunk,
    N=512/matmul); v produced directly in natural [token, dim] layout
    (moving data = wv, N=128) so no PE transpose of v is needed.
  - scores computed transposed per 128-key tile: sT [Tk, Tq], exp on ACT
    straight out of PSUM (scale=1/8 fused, no max pass), causally
    narrowed to the valid Tq range.
  - attn@v in natural orientation: y[tok, hd+1] accumulated over key
    tiles with lhsT = aT tile, rhs = v tile - N=65 moving rows only,
    4x fewer PE cycles than the transposed form. A ones-column in v
    accumulates the softmax denominator into psum column 64. All four
    q-tiles of a chunk accumulate in ONE psum bank per head, so group
    flags are bank-level: one start (first matmul) / stop (last).
  - diagonal masking: one constant [128,128] upper-triangular fp16 mask
    multiplied into the diagonal aT sub-block on DVE (the Pool Q7 launch
    latency + sem-blocked in-order queue stalled the diagonal attn@v).
  - normalize: reciprocal of the denominator column + per-q-tile
    broadcast multiply (DVE, psum->sbuf); fp16 PE transposes (1 cyc/row)
    then give yT [dim, tok] for the output projection, drained by ACT.
  - k-bias is softmax-invariant (adds a per-query constant) - dropped.
    v-bias commutes through softmax (weights sum to 1): folded into the
    host-side output bias as b_attn_v @ w_proj. Only the q-bias is
    applied on-device (per-partition DVE add during the psum drain).
  - software-pipelined emission: each attention chunk's tk-loop trails
    attn@v two tiles behind scores/exp, and weaves in the NEXT chunk's
    qkv units plus the PREVIOUS chunk's normalize/transpose/out-proj
    units as PE filler, so PE never waits on ACT exp or DVE drains.
    Output-projection psum drains split DVE/ACT; out partials are fp16.

Schedule findings from this session (TimelineSim-driven, 125659 -> 121200 ns):
  - the single HWDGE serializes descriptor generation 625ns/DMA and each
    DMA pays ~1.5us fixed latency (dge delay + sem prop), so the startup
    wants FEW, demand-ordered DMAs: bq+wq+wk ride in ONE interleaved
    'wqkb' tensor (bq bitcast as 2 f16 cols; per-kt wq|wk blocks so a
    kt-pair slice is a contiguous >=512B row run), loaded in two slices.
  - startup computes chunk(0,0) q/k kt-pair-major so each 256KB xt slice
    immediately feeds 4 matmuls; v runs after (full-xt dependent).
  - yT2 transpose drains run on ACT only while ACT has slack (steps <
    ytc_s0=4 and the final step); late steps keep ACT exp-only, since the
    tail is ACT-bound (exp is 0.833ns/elem + ~370ns access init, and only
    ACT can exp).
  - final chunk: one merged normalize (recips first, multiplies q01
    before q23 so the first transpose unblocks early), BOTH transpose
    drains on DVE (ytc_final=False: frees ACT to start the fo copies
    ~0.4us earlier in the ACT-serialized tail), transposes before all four
    out-projections, fo1/fo3 psum from the dead psyA/psyB banks so the
    4 out-proj matmuls don't wait on the psP drain rotation. The last
    ~4us is the out-DMA chain (issue+dge+transfer+sem), at its floor.
  - the PE wait-queue allows limited sem-based overtaking (scores ran
    ahead of DMA-blocked v/qkv in the trace), but exploiting it with
    pre-emitted scores (pre00) still measured WORSE.
  - things that LOSE (all measured): fp16-psum scores (bass forbids),
    pairing exps (no psum), psy bank-split / early-finish (+2.6us mid
    cascade), full scores-one-step-ahead pipeline (+12us: every step
    becomes ACT-bound on the next chunk's exp via the psS rotation),
    exp bucket-brigade, qkv quota re-spreading, SWDGE weight loads,
    per-nh out-DMA splits, merged pair out-DMAs, chunk reordering
    (every chunk drags a ~5us finish tail; last-chunk exp depth is not
    the tail driver), and the DMA xbar transpose for yT (+4.6 to +21us:
    despite 14ns/tile transfer cost it head-blocks the in-order SP issue
    queue on its norm dependency, delaying every xt/osb DMA behind it,
    and pays 625ns HWDGE + ~2.9us latency per issue).
"""

import sys

if "/opt/trn_rl_repo" not in sys.path:
    sys.path.insert(0, "/opt/trn_rl_repo")

import numpy as np

import concourse.bass as bass
import concourse.mybir as mybir
import concourse.tile as tile
from concourse import bacc
from concourse.bass import ds, ts
from concourse.bass_utils import run_bass_kernel_spmd

F16 = mybir.dt.float16
F32 = mybir.dt.float32
R32 = mybir.dt.float32r
U16 = mybir.dt.uint16
EXP = mybir.ActivationFunctionType.Exp
ADD = mybir.AluOpType.add
MUL = mybir.AluOpType.mult

N_CORES = 8
HD = 64   # head dim
JW = 128  # per-core qkv width: 2 heads x 64


CFG = {"weave": "jit", "ytp_pool": "psP", "fp8_out": False,
       "trail": 4, "trail_big": 5, "act_mod": 0, "fo_split": False, "atp_bufs": 6,
       "act_last": True, "fp8_v": False, "ytc_act": True,
       "exp_pair": False, "psy_split": False, "fin01_inline": False,
       "defer_s0": 99, "pipe2": False, "atp2_bufs": 20, "pre_n": 0,
       "y2p_bufs": 2, "ytp_bufs": 2, "ytc_s0": 4, "boot2": True,
       "fo_psy_last": True, "dma_split_last": False, "pre_weave": False,
       "norm_merge": False, "ytc_final": False, "boot_tail": "b",
       "qkv_shift": 0,
       "qkv_quota": None}
F8 = mybir.dt.float8e4
DR = mybir.MatmulPerfMode.DoubleRow
OSCALE = 1.0 / 64.0  # wp x16, y x4 on device; undo in the psum drain


UNIT_LOG = []


def _mark(nc, label):
    if CFG.get("log_units"):
        UNIT_LOG.append((label, int(nc.next_id())))


def build_program(B=2, T=2048, C=1024):
    assert T % 512 == 0 and C % 128 == 0
    NCH = T // 512   # 512-token chunks per batch
    KT = C // 128    # contraction tiles for the qkv projection
    NTK = T // 128   # 128-key tiles per batch

    nc = bacc.Bacc("TRN2", target_bir_lowering=False, debug=False)
    xt = nc.dram_tensor("xt", [C, B * T], F16, kind="ExternalInput").ap()
    if CFG["fp8_v"]:
        xt8 = nc.dram_tensor("xt8", [C, B * T], F8, kind="ExternalInput").ap()
        wv8 = nc.dram_tensor("wv8", [64, KT * 2 * JW], F8,
                             kind="ExternalInput").ap()
    else:
        xt8 = wv8 = None
    # weights pre-packed on host: row p holds [kt, 128] contiguous
    if CFG["boot2"] and not CFG["fp8_v"] and not CFG["fp8_out"]:
        # bq (bitcast as 2 f16 cols) + per-kt interleaved wq|wk blocks:
        # one tensor so the startup loads it in two DMAs
        wqkb = nc.dram_tensor("wqkb", [128, 2 + KT * 2 * JW], F16,
                              kind="ExternalInput").ap()
        wq = wk = bq = None
    else:
        wqkb = None
        wq = nc.dram_tensor("wq", [128, KT * JW], F16, kind="ExternalInput").ap()
        wk = nc.dram_tensor("wk", [128, KT * JW], F16, kind="ExternalInput").ap()
        bq = nc.dram_tensor("bq", [JW, 1], F32, kind="ExternalInput").ap()
    wv = nc.dram_tensor("wv", [128, KT * JW], F16, kind="ExternalInput").ap()
    if CFG["fp8_out"]:
        # DoubleRow layout: dim d of the JW contraction lives at
        # (partition d % 64, k-tile d // 64); host packs wp to match.
        wp = nc.dram_tensor("wp", [64, 2 * C], F8, kind="ExternalInput").ap()
    else:
        wp = nc.dram_tensor("wp", [JW, C], F16, kind="ExternalInput").ap()
    out = nc.dram_tensor("out", [B * T, C], F16, kind="ExternalOutput").ap()
    out32 = None  # psum->dram direct ship is impossible: dma_start forbids PSUM src

    xt_r = xt.rearrange("(kt p) t -> p kt t", p=128)
    xt8_r = xt8.rearrange("(kt t p) tok -> p kt t tok", p=64, t=2) if xt8 is not None else None

    with tile.TileContext(nc) as tc:
        _build(tc, B, T, C, NCH, KT, NTK, xt_r, wq, wk, wv, bq, wp, out,
               xt8_r, wv8, wqkb, out32)
    nc.compile()
    return nc


def _build(tc, B, T, C, NCH, KT, NTK, xt_r, wq, wk, wv, bq, wp, out,
           xt8_r=None, wv8=None, wqkb=None, out32=None):
    nc = tc.nc
    from contextlib import ExitStack

    from concourse import library_config

    nc.gpsimd.load_library(library_config.attn)

    with ExitStack() as ctx:
        const = ctx.enter_context(tc.tile_pool(name="const", bufs=1))
        wpool = ctx.enter_context(tc.tile_pool(name="wpool", bufs=1))
        pbp = ctx.enter_context(tc.tile_pool(name="pbp", bufs=1))
        xtp = ctx.enter_context(tc.tile_pool(name="xtp", bufs=CFG.get("xtp_bufs", 2)))
        atp_bufs = (CFG["atp2_bufs"] if CFG["pipe2"]
                    else CFG["atp_bufs"] + CFG["pre_n"])
        atp = ctx.enter_context(tc.tile_pool(name="atp", bufs=atp_bufs))
        y2p = ctx.enter_context(tc.tile_pool(name="y2p", bufs=CFG["y2p_bufs"]))
        ytp = ctx.enter_context(tc.tile_pool(name="ytp", bufs=CFG["ytp_bufs"]))
        rcp = ctx.enter_context(tc.tile_pool(name="rcp", bufs=2))
        osp = ctx.enter_context(tc.tile_pool(name="osp", bufs=CFG.get("osp_bufs", 4)))
        psS = ctx.enter_context(tc.tile_pool(name="psS", bufs=2, space="PSUM"))
        psY = ctx.enter_context(tc.tile_pool(name="psY", bufs=1, space="PSUM"))
        psP = ctx.enter_context(tc.tile_pool(name="psP", bufs=2, space="PSUM"))

        # constants: transpose identity + upper-triangular causal mask (fp16)
        ident = const.tile([128, 128], R32)
        mask = const.tile([128, 128], F16)
        ident16 = const.tile([128, 128], F16)
        if wqkb is None:
            bq_sb = const.tile([JW, 1], F32)

        def emit_consts():
            nc.gpsimd.memset(ident[:].bitcast(mybir.dt.uint32), 0)
            nc.gpsimd.affine_select(
                out=ident[:], in_=ident[:],
                compare_op=mybir.AluOpType.not_equal, fill=1.0,
                base=0, pattern=[[-1, 128]], channel_multiplier=1,
            )
            # mask[p, c] = 1.0 if c >= p else 0  (valid: query c >= key p)
            nc.gpsimd.memset(mask[:].bitcast(U16), 15360)  # fp16 1.0
            nc.gpsimd.affine_select(
                out=mask[:], in_=mask[:],
                compare_op=mybir.AluOpType.is_ge, fill=0.0,
                base=0, pattern=[[1, 128]], channel_multiplier=-1,
            )
            nc.gpsimd.memset(ident16[:].bitcast(U16), 0)
            nc.gpsimd.affine_select(
                out=ident16[:], in_=ident16[:],
                compare_op=mybir.AluOpType.not_equal, fill=1.0,
                base=0, pattern=[[-1, 128]], channel_multiplier=1,
            )
            for _b in range(B):
                nc.gpsimd.memset(
                    vsbs[_b][:, :, :, HD : HD + 1].bitcast(U16), 15360)

        if wqkb is not None:
            wqkb_sb = wpool.tile([128, 2 + KT * 2 * JW], F16)
            bq_ref = wqkb_sb[:, 0:2].bitcast(F32)
            wqk_r = wqkb_sb[:, 2:].rearrange(
                "p (kt two j) -> p kt two j", kt=KT, two=2)

            def wq_kt(kt):
                return wqk_r[:, kt, 0]

            def wk_kt(kt):
                return wqk_r[:, kt, 1]
        else:
            wq_sb = wpool.tile([128, KT, JW], F16)
            wk_sb = wpool.tile([128, KT, JW], F16)
            bq_ref = None

            def wq_kt(kt):
                return wq_sb[:, kt]

            def wk_kt(kt):
                return wk_sb[:, kt]
        if CFG["fp8_v"]:
            wv_sb = wpool.tile([64, KT, 2, JW], F8)
        else:
            wv_sb = wpool.tile([128, KT, JW], F16)
        if CFG["fp8_out"]:
            wp_sb = wpool.tile([64, 2, C], F8)
        else:
            wp_sb = wpool.tile([JW, C], F16)

        # persistent per-batch tensors
        qTs, kTs, vsbs = {}, {}, {}
        for b in range(B):
            qT = pbp.tile([JW, T], F16, tag=f"qT{b}")
            kT = pbp.tile([JW, T], F16, tag=f"kT{b}")
            # v natural layout: [tok-in-tile, key tile, head, hd + ones col]
            vsb = pbp.tile([128, NTK, 2, HD + 1], F16, tag=f"vsb{b}")
            qTs[b], kTs[b], vsbs[b] = qT, kT, vsb

        def qkv_units(b, j, pre_xt=None):
            """qkv projection for 512-token chunk j of batch b, as ~1us
            emission units so it can weave into an attention tk-loop."""
            col0 = b * T + 512 * j
            st = {}

            def u_load_q03():
                _mark(nc, f'qkv{b}.{j}:q03')
                if pre_xt is not None:
                    xt_t = pre_xt
                else:
                    xt_t = xtp.tile([128, KT, 512], F16, tag="xt")
                    if j == 0:
                        for kk in range(0, KT, 2):
                            nc.sync.dma_start(xt_t[:, kk : kk + 2],
                                              xt_r[:, kk : kk + 2, ds(col0, 512)])
                    else:
                        nc.sync.dma_start(xt_t[:, 0:4],
                                          xt_r[:, 0:4, ds(col0, 512)])
                        nc.sync.dma_start(xt_t[:, 4:KT],
                                          xt_r[:, 4:KT, ds(col0, 512)])
                st["xt"] = xt_t
                if CFG["fp8_v"]:
                    xt8_t = xtp.tile([64, KT, 2, 512], F8, tag="xt8")
                    nc.sync.dma_start(xt8_t[:],
                                      xt8_r[:, :, :, ds(col0, 512)])
                    st["xt8"] = xt8_t
                psq = psP.tile([128, 512], F32, tag="p")
                st["psq"] = psq
                for kt in range(4):
                    nc.tensor.matmul(psq[:], wq_kt(kt), xt_t[:, kt],
                                     start=(kt == 0), stop=False)

            def u_q47():
                _mark(nc, f'qkv{b}.{j}:q47')
                xt_t, psq = st["xt"], st["psq"]
                for kt in range(4, KT):
                    nc.tensor.matmul(psq[:], wq_kt(kt), xt_t[:, kt],
                                     start=False, stop=(kt == KT - 1))
                bqv = bq_ref if bq_ref is not None else bq_sb[:]
                nc.vector.tensor_tensor(qTs[b][:, ts(j, 512)], psq[:],
                                        bqv.to_broadcast([JW, 512]), ADD)

            def u_k03():
                _mark(nc, f'qkv{b}.{j}:k03')
                psk = psP.tile([128, 512], F32, tag="p")
                st["psk"] = psk
                for kt in range(4):
                    nc.tensor.matmul(psk[:], wk_kt(kt), st["xt"][:, kt],
                                     start=(kt == 0), stop=False)

            def u_k47():
                _mark(nc, f'qkv{b}.{j}:k47')
                psk = st["psk"]
                for kt in range(4, KT):
                    nc.tensor.matmul(psk[:], wk_kt(kt), st["xt"][:, kt],
                                     start=False, stop=(kt == KT - 1))
                nc.vector.tensor_copy(kTs[b][:, ts(j, 512)], psk[:])

            def u_v01():
                _mark(nc, f'qkv{b}.{j}:v01')
                psv = psP.tile([128, 4, 128], F32, tag="p")
                st["psv"] = psv
                for t4 in range(2):
                    for kt in range(KT):
                        if CFG["fp8_v"]:
                            nc.tensor.matmul(psv[:, t4, :],
                                             st["xt8"][0:64, kt, :, ts(t4, 128)],
                                             wv_sb[0:64, kt, :, :],
                                             start=(t4 == 0 and kt == 0),
                                             stop=False, perf_mode=DR)
                        else:
                            nc.tensor.matmul(psv[:, t4, :],
                                             st["xt"][:, kt, ts(t4, 128)],
                                             wv_sb[:, kt],
                                             start=(t4 == 0 and kt == 0),
                                             stop=False)

            def u_v23():
                _mark(nc, f'qkv{b}.{j}:v23')
                psv = st["psv"]
                for t4 in range(2, 4):
                    for kt in range(KT):
                        if CFG["fp8_v"]:
                            nc.tensor.matmul(psv[:, t4, :],
                                             st["xt8"][0:64, kt, :, ts(t4, 128)],
                                             wv_sb[0:64, kt, :, :],
                                             start=False,
                                             stop=(t4 == 3 and kt == KT - 1),
                                             perf_mode=DR)
                        else:
                            nc.tensor.matmul(psv[:, t4, :],
                                             st["xt"][:, kt, ts(t4, 128)],
                                             wv_sb[:, kt],
                                             start=False,
                                             stop=(t4 == 3 and kt == KT - 1))
                if CFG["fp8_v"]:
                    nc.vector.tensor_scalar(
                        vsbs[b][:, ds(4 * j, 4), :, 0:HD],
                        psv[:].rearrange("p t4 (h d) -> p t4 h d", h=2),
                        1.0 / 16.0, None, MUL)
                else:
                    nc.vector.tensor_copy(
                        vsbs[b][:, ds(4 * j, 4), :, 0:HD],
                        psv[:].rearrange("p t4 (h d) -> p t4 h d", h=2),
                    )

            return [u_load_q03, u_q47, u_k03, u_k47, u_v01, u_v23]

        # ---- pipe2: scores+exp for chunk s+1 are emitted as filler of step
        # s (one step ahead of their attn@v), so no step ever waits on its
        # own exp and the final step has no ACT work at all. qkv runs two
        # steps ahead to feed the advanced scores.
        aT_store = {}

        def mk_s_units(b, j):
            """One scores+exp+mask unit per key tile of chunk (b, j)."""
            qT, kT = qTs[b], kTs[b]

            def mk(tk):
                def u():
                    _mark(nc, f'att{b}.{j}:s{tk}')
                    c0 = max(0, 128 * tk - 512 * j)
                    pss = psS.tile([128, 2, 512], F32, tag="s", name="pss")
                    for h in range(2):
                        nc.tensor.matmul(
                            pss[:, h, c0:512],
                            kT[ds(HD * h, HD), ts(tk, 128)],
                            qT[ds(HD * h, HD), ds(512 * j + c0, 512 - c0)],
                            start=True, stop=True,
                        )
                    aT = atp.tile([128, 2, 512], F16, tag="aT", name="aT")
                    aT_store[(b, j, tk)] = (aT, None)
                    nc.scalar.activation(aT[:, :, c0:512], pss[:, :, c0:512],
                                         EXP, scale=0.125)
                    if tk >= 4 * j:
                        d = tk - 4 * j
                        for h in range(2):
                            nc.vector.tensor_tensor(
                                aT[:, h, ts(d, 128)], aT[:, h, ts(d, 128)],
                                mask[:], MUL,
                            )
                return u

            return [mk(tk) for tk in range(4 * (j + 1))]

        def attn_step2(b, j, prev_norms, fills, final=False):
            """attn@v for chunk (b, j) (aT tiles precomputed last step),
            paced against `fills`. prev_norms run first: they read the
            previous chunk's psy banks, which this chunk's accumulation
            reuses."""
            vsb = vsbs[b]
            ntk = 4 * (j + 1)
            last01 = 4 * j + 1
            for u in prev_norms:
                u()
            psy01 = psY.tile([128, 2, 2, HD + 1], F32, tag="y01",
                             padded_shape=[128, 2, 2, 128], name="psy01")
            psy23 = psY.tile([128, 2, 2, HD + 1], F32, tag="y23",
                             padded_shape=[128, 2, 2, 128], name="psy23")
            st = {}

            def a_unit(tk):
                _mark(nc, f'att{b}.{j}:a{tk}')
                aT, _ = aT_store.pop((b, j, tk))
                for qq in range(4):
                    qg = 4 * j + qq
                    if qg < tk:
                        continue
                    for h in range(2):
                        psy, qi = (psy01, qq) if qq < 2 else (psy23, qq - 2)
                        nc.tensor.matmul(
                            psy[:, h, qi, :],
                            aT[:, h, ts(qq, 128)], vsb[:, tk, h, :],
                            start=(tk == 0 and qq in (0, 2) and h == 0),
                            stop=(h == 1 and ((qq == 1 and tk == last01)
                                              or (qq == 3 and tk == ntk - 1))),
                        )

            def fu_norm(p0):
                _mark(nc, f'att{b}.{j}:norm{p0}')
                if "y2" not in st:
                    st["y2"] = y2p.tile([128, 4, 2, HD], F16, tag="y2",
                                        name="y2")
                y2 = st["y2"]
                psy = psy01 if p0 == 0 else psy23
                for h in range(2):
                    rc = rcp.tile([128, 2, 1], F32, tag=f"rc{h}")
                    nc.vector.reciprocal(rc[:], psy[:, h, :, HD : HD + 1])
                    nc.vector.tensor_tensor(
                        y2[:, ds(p0, 2), h, :], psy[:, h, :, 0:HD],
                        rc[:].to_broadcast([128, 2, HD]), MUL,
                    )

            def fu_transp(p0):
                _mark(nc, f'att{b}.{j}:transp{p0}')
                if "yT2" not in st:
                    yT2 = ytp.tile([128, 4, 128], F16, tag="yT2", name="yT2")
                    st["yT2"] = yT2
                yT2 = st["yT2"]
                yT2p = psP.tile([128, 2, 128], F16, tag="p", name="yT2p",
                                padded_shape=[128, 2, 512])
                for iq, qq in enumerate((p0, p0 + 1)):
                    nc.tensor.matmul(yT2p[:, iq, :],
                                     st["y2"][:, qq, :, :], ident16[:],
                                     is_transpose=True,
                                     start=(iq == 0), stop=(iq == 1))
                if CFG.get("ytc_act"):
                    nc.scalar.activation(yT2[:, ds(p0, 2), :], yT2p[:],
                                         mybir.ActivationFunctionType.Copy)
                else:
                    nc.vector.tensor_copy(yT2[:, ds(p0, 2), :], yT2p[:])

            def fo(qq):
                _mark(nc, f'att{b}.{j}:fo{qq}')
                row0 = b * T + 512 * j + 128 * qq
                osb = osp.tile([128, C], F16, tag="osb", name="osb")
                for nh in range(2):
                    pso = psP.tile([128, 512], F32, tag="p", name="pso")
                    nc.tensor.matmul(pso[:], st["yT2"][:, qq, :],
                                     wp_sb[:, ts(nh, 512)],
                                     start=True, stop=True)
                    if nh == 1 and CFG.get("act_last") and b == B - 1 and j == NCH - 1:
                        nc.scalar.activation(
                            osb[:, ts(nh, 512)], pso[:],
                            mybir.ActivationFunctionType.Copy)
                    else:
                        nc.vector.tensor_copy(osb[:, ts(nh, 512)], pso[:])
                nc.sync.dma_start(out[ds(row0, 128), :], osb[:])

            inline01 = ([lambda: fu_norm(0), lambda: fu_transp(0),
                         lambda: fo(0), lambda: fo(1)] if final else [])
            if final:
                norms = [lambda: fu_norm(2)]
                rest = [lambda: fu_transp(2), lambda: fo(2), lambda: fo(3)]
            else:
                norms = [lambda: fu_norm(0), lambda: fu_norm(2)]
                rest = [lambda: fu_transp(0), lambda: fo(0), lambda: fo(1),
                        lambda: fu_transp(2), lambda: fo(2), lambda: fo(3)]
            pending = list(fills)
            total_fill = len(pending)
            emitted = 0
            for tk in range(ntk):
                a_unit(tk)
                if final and tk == last01:
                    for u in inline01:
                        u()
                target = -(-total_fill * (tk + 1) // ntk)  # ceil
                while emitted < target and pending:
                    pending.pop(0)()
                    emitted += 1
            for u in pending:
                u()
            return norms, rest

        def attn_step(b, j, extra_units, defer=False, final=False, s_idx=0):
            """One pipeline step: the attention tk-loop for chunk (b, j) with
            qkv units for the next chunk and this chunk's own normalize/
            transpose/output-projection units woven in as PE filler."""
            qT, kT, vsb = qTs[b], kTs[b], vsbs[b]
            ntk = 4 * (j + 1)
            last01 = 4 * j + 1  # last key tile contributing to q-tiles 0,1
            if CFG["psy_split"]:
                # q-tiles 0,1 and 2,3 in separate psum banks: the 0,1 group
                # stops at tk=last01 so its normalize/transpose/out-proj can
                # overlap the remaining key tiles' attn@v.
                psy01 = psY.tile([128, 2, 2, HD + 1], F32, tag="y01",
                                 padded_shape=[128, 2, 2, 128], name="psy01")
                psy23 = psY.tile([128, 2, 2, HD + 1], F32, tag="y23",
                                 padded_shape=[128, 2, 2, 128], name="psy23")
            else:
                psyA = psY.tile([128, 4, HD + 1], F32, tag="yA",
                                padded_shape=[128, 4, 128])
                psyB = psY.tile([128, 4, HD + 1], F32, tag="yB",
                                padded_shape=[128, 4, 128])
            st = {}

            aTs = {}

            def s_unit2(m):
                _mark(nc, f'att{b}.{j}:s2_{m}')
                """scores + exp + diagonal masks for key tiles 2m, 2m+1.
                One fp16-psum tile and ONE exp for the pair: the activation's
                ~370ns fixed access cost is paid once per two key tiles. For
                a diagonal pair, tk=2m+1's columns c0a:c0b hold exp of stale
                psum - never read (a_unit skips query blocks < key block)."""
                tka = 2 * m
                c0a = max(0, 128 * tka - 512 * j)
                pss = psS.tile([128, 2, 2, 512], F16, tag="s", name="pss")
                for i in range(2):
                    tk = tka + i
                    c0 = max(0, 128 * tk - 512 * j)
                    for h in range(2):
                        nc.tensor.matmul(
                            pss[:, i, h, c0:512],
                            kT[ds(HD * h, HD), ts(tk, 128)],
                            qT[ds(HD * h, HD), ds(512 * j + c0, 512 - c0)],
                            start=(h == 0), stop=(h == 1),
                        )
                aT = atp.tile([128, 2, 2, 512], F16, tag="aT", name="aT")
                aT_store[(b, j, tka)] = (aT, 0)
                aT_store[(b, j, tka + 1)] = (aT, 1)
                nc.scalar.activation(aT[:, :, :, c0a:512],
                                     pss[:, :, :, c0a:512], EXP, scale=0.125)
                for i in range(2):
                    tk = tka + i
                    if tk >= 4 * j:
                        d = tk - 4 * j  # diagonal q-tile index within chunk
                        for h in range(2):
                            nc.vector.tensor_tensor(
                                aT[:, i, h, ts(d, 128)],
                                aT[:, i, h, ts(d, 128)], mask[:], MUL,
                            )

            def s_unit(tk):
                """scores + exp + diagonal mask for key tile tk."""
                if (b, j, tk) in aT_store:
                    return  # precomputed in an earlier step
                _mark(nc, f'att{b}.{j}:s{tk}')
                c0 = max(0, 128 * tk - 512 * j)
                pss = psS.tile([128, 2, 512], F32, tag="s")
                for h in range(2):
                    nc.tensor.matmul(
                        pss[:, h, c0:512],
                        kT[ds(HD * h, HD), ts(tk, 128)],
                        qT[ds(HD * h, HD), ds(512 * j + c0, 512 - c0)],
                        start=True, stop=True,
                    )
                aT = atp.tile([128, 2, 512], F16, tag="aT")
                aT_store[(b, j, tk)] = (aT, None)
                nc.scalar.activation(aT[:, :, c0:512], pss[:, :, c0:512],
                                     EXP, scale=0.125)
                if tk >= 4 * j:
                    d = tk - 4 * j  # diagonal q-tile index within chunk
                    # DVE, not gpsimd: the Pool Q7 launch latency and its
                    # sem-blocked in-order queue stall the diagonal attn@v
                    for h in range(2):
                        nc.vector.tensor_tensor(
                            aT[:, h, ts(d, 128)], aT[:, h, ts(d, 128)],
                            mask[:], MUL,
                        )

            def a_unit(tk):
                _mark(nc, f'att{b}.{j}:a{tk}')
                """attn@v accumulation for key tile tk (runs one iteration
                behind s_unit so the exp has left the ACT queue)."""
                aT, i = aT_store.pop((b, j, tk))
                for qq in CFG.get("qq_order", (0, 1, 2, 3)):
                    qg = 4 * j + qq  # global q-tile index
                    if qg < tk:
                        continue
                    for h in range(2):
                        ab = (aT[:, i, h, ts(qq, 128)] if i is not None
                              else aT[:, h, ts(qq, 128)])
                        if CFG["psy_split"]:
                            psy, qi = (psy01, qq) if qq < 2 else (psy23, qq - 2)
                            nc.tensor.matmul(
                                psy[:, h, qi, :], ab, vsb[:, tk, h, :],
                                start=(tk == 0 and qq in (0, 2) and h == 0),
                                stop=(h == 1 and ((qq == 1 and tk == last01)
                                                  or (qq == 3 and tk == ntk - 1))),
                            )
                        else:
                            psy = psyA if h == 0 else psyB
                            nc.tensor.matmul(
                                psy[:, qq, :], ab, vsb[:, tk, h, :],
                                start=(tk == 0 and qq == 0),
                                stop=(tk == ntk - 1 and qq == 3),
                            )

            rcs = {}

            def fu_normA(p0):
                """half of the merged normalize: p0=0 also does both
                reciprocals; emitted interleaved with the transposes so the
                DVE in-order queue releases transp0's copy before the q23
                multiplies run (the ACT fo-copy chain starts earlier)."""
                _mark(nc, f'att{b}.{j}:normA{p0}')
                if "y2" not in st:
                    st["y2"] = y2p.tile([128, 4, 2, HD], F16, tag="y2", name="y2")
                y2 = st["y2"]
                if p0 == 0:
                    for h, psy in ((0, psyA), (1, psyB)):
                        rc = rcp.tile([128, 4, 1], F32, tag=f"rca{h}")
                        nc.vector.reciprocal(rc[:], psy[:, :, HD : HD + 1])
                        rcs[h] = rc
                for h, psy in ((0, psyA), (1, psyB)):
                    nc.vector.tensor_tensor(
                        y2[:, ds(p0, 2), h, :], psy[:, ds(p0, 2), 0:HD],
                        rcs[h][:, ds(p0, 2)].to_broadcast([128, 2, HD]),
                        MUL,
                    )

            def fu_norm_all():
                fu_normA(0)
                fu_normA(2)

            def fu_norm(p0):
                _mark(nc, f'att{b}.{j}:norm{p0}')
                """normalize q-tiles p0, p0+1 (attn@v chains stopped): DVE."""
                if "y2" not in st:
                    st["y2"] = y2p.tile([128, 4, 2, HD], F16, tag="y2", name="y2")
                y2 = st["y2"]
                if CFG["psy_split"]:
                    psy = psy01 if p0 == 0 else psy23
                    for h in range(2):
                        rc = rcp.tile([128, 2, 1], F32, tag=f"rc{h}")
                        nc.vector.reciprocal(rc[:], psy[:, h, :, HD : HD + 1])
                        nc.vector.tensor_tensor(
                            y2[:, ds(p0, 2), h, :], psy[:, h, :, 0:HD],
                            rc[:].to_broadcast([128, 2, HD]), MUL,
                        )
                    return
                for h, psy in ((0, psyA), (1, psyB)):
                    rc = rcp.tile([128, 2, 1], F32, tag=f"rc{h}")
                    nc.vector.reciprocal(rc[:], psy[:, ds(p0, 2), HD : HD + 1])
                    nc.vector.tensor_tensor(
                        y2[:, ds(p0, 2), h, :], psy[:, ds(p0, 2), 0:HD],
                        rc[:].to_broadcast([128, 2, HD]), MUL,
                    )

            def fu_transp(p0):
                _mark(nc, f'att{b}.{j}:transp{p0}')
                """transpose q-tiles p0, p0+1 to yT layout. Non-final chunks
                use the DMA xbar transpose (14ns/16x128 tile): no PE matmuls,
                no psum round-trip, no ACT/DVE drain copy - the ~2.5us DMA
                latency is hidden because the finish weaves into the next
                step. The final chunk keeps the low-latency PE path."""
                if (CFG.get("transp_dma") and not final
                        and s_idx < CFG.get("transp_dma_s1", 99)
                        and not CFG["fp8_out"]):
                    if "yT2" not in st:
                        st["yT2"] = ytp.tile([128, 4, 128], F16, tag="yT2",
                                             name="yT2")
                    for qq in (p0, p0 + 1):
                        nc.sync.dma_start_transpose(
                            st["yT2"][:, qq, :], st["y2"][:, qq, :, :])
                    return
                if CFG["fp8_out"]:
                    # split transposes land both JW halves on partitions
                    # 0..63, giving the [64, ktile, tok] DoubleRow layout
                    if "yT2" not in st:
                        st["yT2"] = ytp.tile([64, 4, 2, 128], F8, tag="yT2", name="yT28")
                    yT28 = st["yT2"]
                    yT2p8 = psP.tile([64, 2, 2, 128], R32, tag="p", name="yT2p8")
                    for iq, qq in enumerate((p0, p0 + 1)):
                        for t in range(2):
                            nc.tensor.matmul(
                                yT2p8[0:64, iq, t, :],
                                st["y2"][:, qq, t, :], ident[:],
                                is_transpose=True,
                                start=(iq == 0 and t == 0),
                                stop=(iq == 1 and t == 1))
                    nc.vector.tensor_scalar(
                        yT28[0:64, ds(p0, 2), :, :], yT2p8[0:64],
                        4.0, None, MUL)
                    return
                if "yT2" not in st:
                    yT2 = ytp.tile([128, 4, 128], F16, tag="yT2")
                    st["yT2"] = yT2
                yT2 = st["yT2"]
                yT2p = psP.tile([128, 2, 128], F16, tag="p", name="yT2p",
                                padded_shape=[128, 2, 512])
                for iq, qq in enumerate((p0, p0 + 1)):
                    nc.tensor.matmul(yT2p[:, iq, :],
                                     st["y2"][:, qq, :, :], ident16[:],
                                     is_transpose=True,
                                     start=(iq == 0), stop=(iq == 1))
                use_act = CFG.get("ytc_act") and (
                    s_idx < CFG["ytc_s0"]
                    or (final and CFG.get("ytc_final", True)))
                if final and CFG.get("ytc_split_final", True) and p0 == 0:
                    use_act = False  # DVE is free right after the norms
                if use_act:
                    nc.scalar.activation(yT2[:, ds(p0, 2), :], yT2p[:],
                                         mybir.ActivationFunctionType.Copy)
                else:
                    nc.vector.tensor_copy(yT2[:, ds(p0, 2), :], yT2p[:])

            def fo(qq, nhs=(0, 1)):
                _mark(nc, f'att{b}.{j}:fo{qq}')
                row0 = b * T + 512 * j + 128 * qq
                merge = final and CFG.get("fo_merge_last", False)
                if merge:
                    pair = qq // 2
                    if qq % 2 == 0 and 0 in nhs:
                        st[f"osb2_{pair}"] = osp.tile(
                            [128, 2, C], F16, tag="osb2", name="osb2")
                    st[f"osb{qq}"] = st[f"osb2_{pair}"][:, qq % 2]
                elif 0 in nhs:
                    st[f"osb{qq}"] = osp.tile([128, C], F16, tag="osb",
                                              name="osb")
                osb = st[f"osb{qq}"]
                # final chunk: after the norms, psyA/psyB banks are dead -
                # use them as two extra pso buffers so the four out-proj
                # matmuls stream without waiting on the psP drain rotation.
                use_yab = (CFG.get("fo_psy_last") and not CFG["psy_split"]
                           and final and qq in (1, 3))
                use_s = CFG.get("fo_pss_last", False) and final and qq == 2
                raw32 = (CFG.get("fo3_raw32", False) and final and qq == 3
                         and out32 is not None and not CFG["fp8_out"])
                for nh in nhs:
                    if use_yab:
                        pso = psY.tile([128, 512], F32,
                                       tag=("yA" if nh == 0 else "yB"),
                                       name="psoY")
                    elif use_s:
                        pso = psS.tile([128, 512], F32, tag="s", name="psoS")
                    else:
                        pso = psP.tile([128, 512], F32, tag="p")
                    if CFG["fp8_out"]:
                        nc.tensor.matmul(pso[:], st["yT2"][0:64, qq, :, :],
                                         wp_sb[0:64, :, ts(nh, 512)],
                                         start=True, stop=True, perf_mode=DR)
                    else:
                        nc.tensor.matmul(pso[:], st["yT2"][:, qq, :],
                                         wp_sb[:, ts(nh, 512)],
                                         start=True, stop=True)
                    if raw32:
                        # last q-tile: psum straight to dram in fp32, skipping
                        # the drain-copy hop that gates program end (the host
                        # sums partials in fp32 anyway)
                        nc.sync.dma_start(out32[:, ts(nh, 512)], pso[:])
                        continue
    
                    if final and CFG.get("act_alt_last", False):
                        if (qq + nh) % 2 == 0:
                            nc.scalar.activation(
                                osb[:, ts(nh, 512)], pso[:],
                                mybir.ActivationFunctionType.Copy)
                        else:
                            nc.vector.tensor_copy(osb[:, ts(nh, 512)], pso[:])
                        continue
                    if nh == 1 and ((CFG["act_mod"] and (j + qq) % CFG["act_mod"] == 0) or (CFG.get("act_last") and final) or (CFG.get("act_early") and j <= 0)):
                        nc.scalar.activation(
                            osb[:, ts(nh, 512)], pso[:],
                            mybir.ActivationFunctionType.Copy,
                            scale=OSCALE if CFG["fp8_out"] else 1.0,
                        )
                    elif CFG["fp8_out"]:
                        nc.vector.tensor_scalar(
                            osb[:, ts(nh, 512)], pso[:], OSCALE, None, MUL)
                    else:
                        nc.vector.tensor_copy(osb[:, ts(nh, 512)], pso[:])
                    if (CFG.get("dma_split_last") and b == B - 1
                            and j == NCH - 1):
                        nc.sync.dma_start(
                            out[ds(row0, 128), ts(nh, 512)],
                            osb[:, ts(nh, 512)])
                if raw32:
                    return
                if 1 in nhs and merge:
                    if qq % 2 == 1:
                        r0 = (b * T + 512 * j) // 128 + qq - 1
                        out_r = out.rearrange("(r p) c -> p r c", p=128)
                        nc.sync.dma_start(out_r[:, r0 : r0 + 2, :],
                                          st[f"osb2_{qq // 2}"][:])
                elif (1 in nhs and final and qq == 3
                      and CFG.get("dma_split_fo3", False)):
                    # the very last transfer gates program end: halve it so
                    # the nh0 half ships while nh1 still drains
                    nc.sync.dma_start(out[ds(row0, 128), ts(0, 512)],
                                      osb[:, ts(0, 512)])
                    nc.sync.dma_start(out[ds(row0, 128), ts(1, 512)],
                                      osb[:, ts(1, 512)])
                elif 1 in nhs and not (CFG.get("dma_split_last") and final):
                    nc.sync.dma_start(out[ds(row0, 128), :], osb[:])

            # the psum zero-region rule forbids reading psy mid-group, so
            # finish units run after their psum group stops. With psy_split,
            # the q-tiles-0,1 group stops at tk=last01: its normalize runs
            # inline right after (freeing the bank early), and for the final
            # chunk the whole 0,1 finish chain runs inline so the output
            # drains/DMAs overlap the tail key tiles' exp-bound attn@v.
            # `defer` steps push their transpose/out-proj PE work into the
            # final step, where PE otherwise idles behind ACT.
            finishA = [lambda: fu_transp(0), lambda: fo(0), lambda: fo(1)]
            if (final and CFG.get("norm_split_final", True)
                    and not CFG["psy_split"]):
                finishB = [lambda: fu_normA(2), lambda: fu_transp(2),
                           lambda: fo(2), lambda: fo(3)]
                norm01 = [lambda: fu_normA(0), lambda: fu_transp(0),
                          lambda: fo(0), lambda: fo(1)]
                finishA = []
            elif ((CFG.get("norm_merge") or (final and CFG.get("norm_merge_last", True)))
                    and not CFG["psy_split"]):
                finishB = [lambda: fu_transp(2), lambda: fo(2), lambda: fo(3)]
                norm01 = [fu_norm_all]
            else:
                finishB = [lambda: fu_norm(2), lambda: fu_transp(2),
                           lambda: fo(2), lambda: fo(3)]
                norm01 = [lambda: fu_norm(0)]
            # inline norm01 mid-step would park a sem-blocked op at the head
            # of DVE's in-order queue and stall the woven qkv drains behind
            # it - only the final step (no downstream DVE consumers) inlines.
            inline01 = []
            if CFG["psy_split"] and final and CFG["fin01_inline"]:
                inline01 = norm01 + finishA
                carry, deferred = finishB, []
            elif CFG["psy_split"] and defer:
                carry, deferred = norm01 + [finishB[0]], finishA + finishB[1:]
            else:
                carry, deferred = norm01 + finishA + finishB, []
                if (CFG.get("transp_dma") and not final
                        and s_idx < CFG.get("transp_dma_s1", 99)):
                    # DMA transposes need their ~2.9us latency hidden:
                    # issue both early, draw the fo units late
                    carry = (norm01
                             + [lambda: fu_transp(0), lambda: fu_norm(2),
                                lambda: fu_transp(2), lambda: fo(0),
                                lambda: fo(1), lambda: fo(2), lambda: fo(3)])
            pending = list(extra_units)
            total_fill = len(pending)
            emitted = 0
            TRAIL = CFG["trail"] if ntk > 4 else CFG.get("trail_small", 1)
            if CFG.get("trail_big") and ntk >= 12:
                TRAIL = CFG["trail_big"]

            def emit_a(tk):
                a_unit(tk)
                if tk == last01:
                    for u in inline01:
                        u()

            for tk in range(ntk):
                if CFG["exp_pair"]:
                    if tk % 2 == 0:
                        s_unit2(tk // 2)
                else:
                    s_unit(tk)
                if final and CFG.get("fill_pow2_final"):
                    frac = float(tk + 1) / ntk
                    target = int(total_fill * frac * frac + 0.999)
                else:
                    target = -(-total_fill * (tk + 1) // ntk)  # ceil
                nb = CFG.get("fill_before_a", 1)
                while nb > 0 and emitted < target and pending:
                    pending.pop(0)()
                    emitted += 1
                    nb -= 1
                if tk >= TRAIL:
                    emit_a(tk - TRAIL)
                while emitted < target and pending:
                    pending.pop(0)()
                    emitted += 1
            for tk in range(ntk - TRAIL, ntk):
                emit_a(tk)
                if pending:
                    pending.pop(0)()
            for u in pending:
                u()
            return carry, deferred

        # ---- startup: stage DMAs so the first q matmuls start early ----
        if not (CFG["boot2"] and not CFG["fp8_v"] and not CFG["fp8_out"]):
            emit_consts()
        wq_r2 = wq.rearrange("p (kt j) -> p kt j", kt=KT) if wq is not None else None
        xt00 = xtp.tile([128, KT, 512], F16, tag="xt")
        if CFG["boot2"] and not CFG["fp8_v"] and not CFG["fp8_out"]:
            # kt-pair-major startup: q/k of chunk (0,0) consume each 256KB
            # xt slice as it lands, so PE food arrives per-DMA instead of
            # per-projection. The DMA stream is emitted in consumption
            # order (the single HWDGE serializes descriptor generation).
            xt10 = xtp.tile([128, KT, 512], F16, tag="xt", name="xt10")
            W0 = 2 + 2 * 2 * JW  # bq + kt0,kt1 interleaved blocks
            W1 = 2 + 2 * JW      # bq + kt0 block only
            bf = CFG.get("boot_first", "wq")
            if bf == "xt":
                nc.sync.dma_start(xt00[:, 0:2], xt_r[:, 0:2, ds(0, 512)])
                nc.sync.dma_start(wqkb_sb[:, 0:W0], wqkb[:, 0:W0])
            elif bf == "fine":
                nc.sync.dma_start(wqkb_sb[:, 0:W1], wqkb[:, 0:W1])
                nc.sync.dma_start(xt00[:, 0:1], xt_r[:, 0:1, ds(0, 512)])
                nc.sync.dma_start(wqkb_sb[:, W1:W0], wqkb[:, W1:W0])
                nc.sync.dma_start(xt00[:, 1:2], xt_r[:, 1:2, ds(0, 512)])
            else:
                nc.sync.dma_start(wqkb_sb[:, 0:W0], wqkb[:, 0:W0])
                nc.sync.dma_start(xt00[:, 0:2], xt_r[:, 0:2, ds(0, 512)])
            nc.sync.dma_start(wqkb_sb[:, W0:], wqkb[:, W0:])
            nc.sync.dma_start(xt00[:, 2:4], xt_r[:, 2:4, ds(0, 512)])
            tail_order = CFG.get("boot_tail", "a")
            if tail_order == "a":
                nc.sync.dma_start(xt00[:, 4:6], xt_r[:, 4:6, ds(0, 512)])
                nc.sync.dma_start(xt00[:, 6:KT], xt_r[:, 6:KT, ds(0, 512)])
                nc.sync.dma_start(wv_sb[:],
                                  wv.rearrange("p (kt j) -> p kt j", kt=KT))
                nc.sync.dma_start(xt10[:, 0:4], xt_r[:, 0:4, ds(T, 512)])
                nc.sync.dma_start(xt10[:, 4:KT], xt_r[:, 4:KT, ds(T, 512)])
                nc.sync.dma_start(wp_sb[:], wp)
            elif tail_order == "b":  # xt10a before wv
                wv_r = wv.rearrange("p (kt j) -> p kt j", kt=KT)
                nc.sync.dma_start(xt00[:, 4:6], xt_r[:, 4:6, ds(0, 512)])
                nc.sync.dma_start(xt00[:, 6:KT], xt_r[:, 6:KT, ds(0, 512)])
                if CFG.get("v0_kt_outer"):
                    nc.sync.dma_start(wv_sb[:, 0:2], wv_r[:, 0:2])
                    nc.sync.dma_start(xt10[:, 0:4], xt_r[:, 0:4, ds(T, 512)])
                    nc.sync.dma_start(wv_sb[:, 2:KT], wv_r[:, 2:KT])
                else:
                    nc.sync.dma_start(xt10[:, 0:4], xt_r[:, 0:4, ds(T, 512)])
                    nc.sync.dma_start(wv_sb[:], wv_r)
                nc.sync.dma_start(xt10[:, 4:KT], xt_r[:, 4:KT, ds(T, 512)])
                nc.sync.dma_start(wp_sb[:], wp)
            else:  # c: xt00 fully before weights tail
                nc.sync.dma_start(xt00[:, 4:KT], xt_r[:, 4:KT, ds(0, 512)])
                nc.sync.dma_start(wv_sb[:],
                                  wv.rearrange("p (kt j) -> p kt j", kt=KT))
                nc.sync.dma_start(xt10[:, 0:4], xt_r[:, 0:4, ds(T, 512)])
                nc.sync.dma_start(xt10[:, 4:KT], xt_r[:, 4:KT, ds(T, 512)])
                nc.sync.dma_start(wp_sb[:], wp)
            emit_consts()
            psq0 = psP.tile([128, 512], F32, tag="p", name="psq0")
            psk0 = psP.tile([128, 512], F32, tag="p", name="psk0")
            for kp in range(KT // 2):
                for kt in (2 * kp, 2 * kp + 1):
                    nc.tensor.matmul(psq0[:], wq_kt(kt), xt00[:, kt],
                                     start=(kt == 0), stop=(kt == KT - 1))
                for kt in (2 * kp, 2 * kp + 1):
                    nc.tensor.matmul(psk0[:], wk_kt(kt), xt00[:, kt],
                                     start=(kt == 0), stop=(kt == KT - 1))
            bqv0 = bq_ref if bq_ref is not None else bq_sb[:]
            nc.vector.tensor_tensor(qTs[0][:, ts(0, 512)], psq0[:],
                                    bqv0.to_broadcast([JW, 512]), ADD)
            nc.vector.tensor_copy(kTs[0][:, ts(0, 512)], psk0[:])
            # scores+exp for chunk (0,0) here: the PE wait-queue lets them
            # overtake the DMA-blocked v/qkv(1,0) matmuls, so the ACT exp
            # stream starts ~3us earlier in an otherwise idle window
            for u in mk_s_units(0, 0)[:CFG.get("pre00", 0)]:
                u()
            psv0 = psP.tile([128, 4, 128], F32, tag="p", name="psv0")
            if CFG.get("v0_kt_outer"):
                for kt in range(KT):
                    for t4 in range(4):
                        nc.tensor.matmul(psv0[:, t4, :],
                                         xt00[:, kt, ts(t4, 128)],
                                         wv_sb[:, kt],
                                         start=(kt == 0 and t4 == 0),
                                         stop=(kt == KT - 1 and t4 == 3))
            else:
                for t4 in range(4):
                    for kt in range(KT):
                        nc.tensor.matmul(psv0[:, t4, :],
                                         xt00[:, kt, ts(t4, 128)],
                                         wv_sb[:, kt],
                                         start=(t4 == 0 and kt == 0),
                                         stop=(t4 == 3 and kt == KT - 1))
            nc.vector.tensor_copy(
                vsbs[0][:, ds(0, 4), :, 0:HD],
                psv0[:].rearrange("p t4 (h d) -> p t4 h d", h=2))
            u0 = qkv_units(1, 0, pre_xt=xt10)
            for u in u0:
                u()
            boot2_done = True
        else:
            boot2_done = False
            nc.sync.dma_start(wq_sb[:, 0:2], wq_r2[:, 0:2])
            nc.sync.dma_start(xt00[:, 0:2], xt_r[:, 0:2, ds(0, 512)])
            nc.sync.dma_start(wq_sb[:, 2:4], wq_r2[:, 2:4])
            nc.sync.dma_start(xt00[:, 2:4], xt_r[:, 2:4, ds(0, 512)])
            nc.sync.dma_start(wq_sb[:, 4:KT], wq_r2[:, 4:KT])
            nc.sync.dma_start(xt00[:, 4:6], xt_r[:, 4:6, ds(0, 512)])
            nc.sync.dma_start(bq_sb[:], bq)
            nc.sync.dma_start(xt00[:, 6:KT], xt_r[:, 6:KT, ds(0, 512)])
            nc.sync.dma_start(wk_sb[:], wk.rearrange("p (kt j) -> p kt j", kt=KT))
            u0 = qkv_units(0, 0, pre_xt=xt00)
            u0[0]()  # first q matmuls
            u0[1]()
            if CFG["fp8_v"]:
                nc.sync.dma_start(wv_sb[:],
                                  wv8.rearrange("p (kt t j) -> p kt t j",
                                                kt=KT, t=2))
            else:
                nc.sync.dma_start(wv_sb[:],
                                  wv.rearrange("p (kt j) -> p kt j", kt=KT))
            u0[2]()
            u0[3]()
            if CFG["fp8_out"]:
                nc.sync.dma_start(wp_sb[:], wp.rearrange("p (t c) -> p t c", t=2))
            else:
                nc.sync.dma_start(wp_sb[:], wp)
            u0[4]()
            u0[5]()
        steps = [(b, j) for j in range(NCH) for b in range(B)]
        if CFG["pipe2"]:
            # startup tail: qkv(1,0) interleaved with scores(0,0)
            q10 = qkv_units(1, 0)
            s00 = mk_s_units(0, 0)
            seq = []
            for i in range(max(len(q10), len(s00))):
                if i < len(q10):
                    seq.append(q10[i])
                if i < len(s00):
                    seq.append(s00[i])
            for u in seq:
                u()
            norms, rest = [], []
            for s, (b, j) in enumerate(steps):
                final = s == len(steps) - 1
                s_next = mk_s_units(*steps[s + 1]) if s + 1 < len(steps) else []
                qk = qkv_units(*steps[s + 2]) if s + 2 < len(steps) else []
                inter = []
                for i in range(max(len(s_next), len(qk))):
                    if i < len(s_next):
                        inter.append(s_next[i])
                    if i < len(qk):
                        inter.append(qk[i])
                fills = rest + inter
                norms, rest = attn_step2(b, j, norms, fills, final=final)
            for u in norms + rest:
                u()
            return
        if not boot2_done:
            for u in qkv_units(1, 0):
                u()
        # just-in-time weave: qkv for step s+1's chunk fills step s, so the
        # tail steps (which have no qkv of their own left) stay PE-fed
        carried = []
        deferred_pool = []
        S0 = CFG["defer_s0"]
        # step_order: attention chunks may run in any order that satisfies
        # "attn(b,j) after qkv(b, j' <= j)"; ending on a SHALLOW chunk keeps
        # the final (fill-less) step's exp load small. qkv_order[s] names the
        # chunk whose qkv weaves into step s.
        if CFG.get("step_order"):
            steps = [tuple(x) for x in CFG["step_order"]]
            qkv_order = [tuple(x) for x in CFG["qkv_order"]]
        else:
            qkv_order = None
        # qkv fill queue: all remaining chunks' qkv units in step order,
        # drawn per-step by quota so the exp-latency-bound early steps get
        # PE fill too (their own chunks are too shallow to hide exp).
        quota = CFG.get("qkv_quota")
        if quota:
            qkv_queue = []
            for (qb, qj) in steps:
                if qj >= 1:
                    qkv_queue += qkv_units(qb, qj)
        qkv_cache = {}
        for s, (b, j) in enumerate(steps):
            final = s == len(steps) - 1
            if qkv_order is not None:
                qk = (qkv_units(*qkv_order[s])
                      if s < len(qkv_order) else [])
            elif quota:
                take = min(quota[s] if s < len(quota) else 99, len(qkv_queue))
                qk = qkv_queue[:take]
                del qkv_queue[:take]
            elif CFG["weave"] == "jit":
                sh = CFG.get("qkv_shift", 0)
                if sh:
                    # split each chunk's 6 qkv units sh/6-sh across two steps
                    qk = []
                    if s + 1 < len(steps) and steps[s + 1][1] >= 1:
                        qk += qkv_cache.setdefault(
                            steps[s + 1], qkv_units(*steps[s + 1]))[sh:]
                    if s + 2 < len(steps) and steps[s + 2][1] >= 1:
                        qk += qkv_cache.setdefault(
                            steps[s + 2], qkv_units(*steps[s + 2]))[:sh]
                elif s + 1 < len(steps) and steps[s + 1][1] >= 1:
                    qk = qkv_units(*steps[s + 1])
                else:
                    qk = []
            else:
                qk = qkv_units(b, j + 1) if j + 1 < NCH else []
            if final:
                qk = qk + deferred_pool
                deferred_pool = []
            # bucket brigade: precompute the next chunk's first pre_n
            # scores+exp during this step, cascading exp load away from the
            # ACT-bound final steps. Emitted after this step's qkv units
            # (which write the qT/kT they read).
            pre = []
            if CFG["pre_n"] > 0 and s + 1 < len(steps) and steps[s + 1][1] >= 1:
                pre = mk_s_units(*steps[s + 1])[:CFG["pre_n"]]
            # interleave the previous step's finish units with qkv units
            ilv = CFG.get("ilv", 0)
            if ilv == 1:
                extra = qk + carried
            elif ilv == 2:
                extra = carried + qk
            elif ilv == 3:  # qkv leads the alternation
                extra = []
                for i in range(max(len(carried), len(qk))):
                    if i < len(qk):
                        extra.append(qk[i])
                    if i < len(carried):
                        extra.append(carried[i])
            else:
                extra = []
                for i in range(max(len(carried), len(qk))):
                    if i < len(carried):
                        extra.append(carried[i])
                    if i < len(qk):
                        extra.append(qk[i])
            if CFG.get("pre_weave"):
                ex2 = []
                for i, u in enumerate(extra):
                    ex2.append(u)
                    if i >= 2 and i - 2 < len(pre):
                        ex2.append(pre[i - 2])
                extra = ex2 + pre[max(0, len(extra) - 2):]
            else:
                extra = extra + pre
            carried, dfr = attn_step(b, j, extra,
                                     defer=(S0 <= s < len(steps) - 1),
                                     final=final, s_idx=s)
            deferred_pool += dfr
        # tail: run normalize+transpose first so the four output
        # projections stream back-to-back with their DVE/ACT drains parallel
        if CFG.get("norm_split_final", True) and not CFG["psy_split"]:
            # pre-ordered: [nA0, t0, nA2, t2, fo0, fo1, fo2, fo3]
            carried = [carried[i] for i in (0, 1, 4, 5, 2, 3, 6, 7)]
        elif len(carried) == 8:
            carried = [carried[i] for i in (0, 1, 4, 5, 2, 3, 6, 7)]
        elif len(carried) == 7:  # norm_merge: [nA, t0, fo0, fo1, t2, fo2, fo3]
            carried = [carried[i] for i in (0, 1, 4, 2, 3, 5, 6)]
        for u in carried:
            u()


def _pack_wv8(wv):
    # [C, 128] -> [64, KT, 2, 128]: contraction dim c = kt*128 + t*64 + p
    import ml_dtypes
    KT = wv.shape[0] // 128
    w = (16.0 * wv).reshape(KT, 2, 64, 128).transpose(2, 0, 1, 3)
    return np.ascontiguousarray(w.reshape(64, -1).astype(ml_dtypes.float8_e4m3))


def _pack_wp(wp):
    if CFG["fp8_out"]:
        import ml_dtypes
        return np.ascontiguousarray(
            (16.0 * wp.reshape(2, 64, -1).transpose(1, 0, 2)
             .reshape(64, -1)).astype(ml_dtypes.float8_e4m3)
        )
    return np.ascontiguousarray(wp.astype(np.float16))


def make_in_maps(x, w_attn, b_attn, w_proj):
    B, T, C = x.shape
    KT = C // 128
    x = np.asarray(x, np.float32)
    w_attn = np.asarray(w_attn, np.float32)
    b_attn = np.asarray(b_attn, np.float32)
    w_proj = np.asarray(w_proj, np.float32)
    xt = np.ascontiguousarray(
        x.transpose(2, 0, 1).reshape(C, B * T).astype(np.float16)
    )
    if CFG["fp8_v"]:
        import ml_dtypes
        xt8 = np.ascontiguousarray(
            x.transpose(2, 0, 1).reshape(C, B * T).astype(ml_dtypes.float8_e4m3)
        )

    def pack(w):  # [C, 128] -> [128, KT*128], row p = [kt, j] contiguous
        return np.ascontiguousarray(
            w.reshape(KT, 128, 128).transpose(1, 0, 2).reshape(128, KT * 128)
        ).astype(np.float16)

    packed = CFG["boot2"] and not CFG["fp8_v"] and not CFG["fp8_out"]

    def pack_qkb(wqc, wkc, bqc):
        # [128, 2 + KT*2*128]: row p = [bq(f32 as 2xf16) | kt: wq_kt | wk_kt]
        wqk = np.stack(
            [wqc.reshape(KT, 128, 128), wkc.reshape(KT, 128, 128)], axis=1
        ).transpose(2, 0, 1, 3).reshape(128, KT * 2 * 128).astype(np.float16)
        head = np.ascontiguousarray(
            bqc.astype(np.float32).reshape(128, 1)).view(np.float16)
        return np.ascontiguousarray(np.concatenate([head, wqk], axis=1))

    in_maps = []
    for i in range(N_CORES):
        h0 = i * JW
        extra8 = (
            {"xt8": xt8,
             "wv8": _pack_wv8(w_attn[:, 2 * C + h0 : 2 * C + h0 + JW])}
            if CFG["fp8_v"] else {}
        )
        if packed:
            wqk_entries = {
                "wqkb": pack_qkb(w_attn[:, h0 : h0 + JW],
                                 w_attn[:, C + h0 : C + h0 + JW],
                                 b_attn[h0 : h0 + JW]),
            }
        else:
            wqk_entries = {
                "wq": pack(w_attn[:, h0 : h0 + JW]),
                "wk": pack(w_attn[:, C + h0 : C + h0 + JW]),
                "bq": np.ascontiguousarray(
                    b_attn[h0 : h0 + JW].reshape(JW, 1).astype(np.float32)
                ),
            }
        in_maps.append(
            {
                **extra8,
                **wqk_entries,
                "xt": xt,
                "wv": pack(w_attn[:, 2 * C + h0 : 2 * C + h0 + JW]),
                "wp": _pack_wp(w_proj[h0 : h0 + JW, :]),
            }
        )
    return in_maps


_PROGRAM_CACHE = {}


def _get_program(B, T, C):
    key = (B, T, C)
    if key not in _PROGRAM_CACHE:
        _PROGRAM_CACHE[key] = build_program(B, T, C)
    return _PROGRAM_CACHE[key]


def kernel(x, w_attn, b_attn, w_proj, b_proj, _trace=False):
    B, T, C = x.shape
    nc = _get_program(B, T, C)
    in_maps = make_in_maps(x, w_attn, b_attn, w_proj)
    res = run_bass_kernel_spmd(nc, in_maps, list(range(N_CORES)), trace=_trace)
    out = np.zeros((B * T, C), np.float32)
    for r in res.results:
        part = np.asarray(r["out"], np.float32)
        if CFG.get("fo3_raw32", False):
            # last q-tile shipped as raw fp32 psum; its fp16 rows are unwritten
            part = part.copy()
            part[B * T - 128 :] = np.asarray(r["out32"], np.float32)
        out += part
    b_attn = np.asarray(b_attn, np.float32)
    w_proj = np.asarray(w_proj, np.float32)
    # v-bias commutes through softmax; k-bias is softmax-invariant
    out += (np.asarray(b_proj, np.float32) + b_attn[2 * C :] @ w_proj)[None, :]
    out = out.reshape(B, T, C)
    kernel.last_exec_time_ns = res.exec_time_ns
    return out



# revision 77
# speedup vs baseline: 1.0019x; 1.0019x over previous
"""Causal self-attention (B=2, T=2048, C=1024, H=16) on 8 trn2 NeuronCores.

Sharding: tensor-parallel over heads - 2 heads per core. Each core computes
its heads' qkv projection (column-split w_attn), causal attention, and a
row-split partial of the output projection; the host sums the 8 fp16
partials and adds the biases.

Design notes (driven by the TimelineSim cost model, which bills a matmul
as moving-rows x cycle regardless of K/M):
  - fp16 everywhere on the PE (1 cyc/row unconditionally; fp32r pays 4x
    below 256 moving rows; fp8 tested and rejected: 3e-2 rel err).
  - q,k produced transposed (qT/kT [128, T], moving data = x chunk,
    N=512/matmul); v produced directly in natural [token, dim] layout
    (moving data = wv, N=128) so no PE transpose of v is needed.
  - scores computed transposed per 128-key tile: sT [Tk, Tq], exp on ACT
    straight out of PSUM (scale=1/8 fused, no max pass), causally
    narrowed to the valid Tq range.
  - attn@v in natural orientation: y[tok, hd+1] accumulated over key
    tiles with lhsT = aT tile, rhs = v tile - N=65 moving rows only,
    4x fewer PE cycles than the transposed form. A ones-column in v
    accumulates the softmax denominator into psum column 64. All four
    q-tiles of a chunk accumulate in ONE psum bank per head, so group
    flags are bank-level: one start (first matmul) / stop (last).
  - diagonal masking: one constant [128,128] upper-triangular fp16 mask
    multiplied into the diagonal aT sub-block on DVE (the Pool Q7 launch
    latency + sem-blocked in-order queue stalled the diagonal attn@v).
  - normalize: reciprocal of the denominator column + per-q-tile
    broadcast multiply (DVE, psum->sbuf); fp16 PE transposes (1 cyc/row)
    then give yT [dim, tok] for the output projection, drained by ACT.
  - k-bias is softmax-invariant (adds a per-query constant) - dropped.
    v-bias commutes through softmax (weights sum to 1): folded into the
    host-side output bias as b_attn_v @ w_proj. Only the q-bias is
    applied on-device (per-partition DVE add during the psum drain).
  - software-pipelined emission: each attention chunk's tk-loop trails
    attn@v two tiles behind scores/exp, and weaves in the NEXT chunk's
    qkv units plus the PREVIOUS chunk's normalize/transpose/out-proj
    units as PE filler, so PE never waits on ACT exp or DVE drains.
    Output-projection psum drains split DVE/ACT; out partials are fp16.

Schedule findings from this session (TimelineSim-driven, 125659 -> 121200 ns):
  - the single HWDGE serializes descriptor generation 625ns/DMA and each
    DMA pays ~1.5us fixed latency (dge delay + sem prop), so the startup
    wants FEW, demand-ordered DMAs: bq+wq+wk ride in ONE interleaved
    'wqkb' tensor (bq bitcast as 2 f16 cols; per-kt wq|wk blocks so a
    kt-pair slice is a contiguous >=512B row run), loaded in two slices.
  - startup computes chunk(0,0) q/k kt-pair-major so each 256KB xt slice
    immediately feeds 4 matmuls; v runs after (full-xt dependent).
  - yT2 transpose drains run on ACT only while ACT has slack (steps <
    ytc_s0=4 and the final step); late steps keep ACT exp-only, since the
    tail is ACT-bound (exp is 0.833ns/elem + ~370ns access init, and only
    ACT can exp).
  - final chunk: one merged normalize (recips first, multiplies q01
    before q23 so the first transpose unblocks early), BOTH transpose
    drains on DVE (ytc_final=False: frees ACT to start the fo copies
    ~0.4us earlier in the ACT-serialized tail), transposes before all four
    out-projections, fo1/fo3 psum from the dead psyA/psyB banks so the
    4 out-proj matmuls don't wait on the psP drain rotation. The last
    ~4us is the out-DMA chain (issue+dge+transfer+sem), at its floor.
  - the PE wait-queue allows limited sem-based overtaking (scores ran
    ahead of DMA-blocked v/qkv in the trace), but exploiting it with
    pre-emitted scores (pre00) still measured WORSE.
  - things that LOSE (all measured): fp16-psum scores (bass forbids),
    pairing exps (no psum), psy bank-split / early-finish (+2.6us mid
    cascade), full scores-one-step-ahead pipeline (+12us: every step
    becomes ACT-bound on the next chunk's exp via the psS rotation),
    exp bucket-brigade, qkv quota re-spreading, SWDGE weight loads,
    per-nh out-DMA splits, merged pair out-DMAs, chunk reordering
    (every chunk drags a ~5us finish tail; last-chunk exp depth is not
    the tail driver), and the DMA xbar transpose for yT (+4.6 to +21us:
    despite 14ns/tile transfer cost it head-blocks the in-order SP issue
    queue on its norm dependency, delaying every xt/osb DMA behind it,
    and pays 625ns HWDGE + ~2.9us latency per issue).
"""

import sys

if "/opt/trn_rl_repo" not in sys.path:
    sys.path.insert(0, "/opt/trn_rl_repo")

import numpy as np

import concourse.bass as bass
import concourse.mybir as mybir
import concourse.tile as tile
from concourse import bacc
from concourse.bass import ds, ts
from concourse.bass_utils import run_bass_kernel_spmd

F16 = mybir.dt.float16
F32 = mybir.dt.float32
R32 = mybir.dt.float32r
U16 = mybir.dt.uint16
EXP = mybir.ActivationFunctionType.Exp
ADD = mybir.AluOpType.add
MUL = mybir.AluOpType.mult

N_CORES = 8
HD = 64   # head dim
JW = 128  # per-core qkv width: 2 heads x 64


CFG = {"weave": "jit", "ytp_pool": "psP", "fp8_out": False,
       "trail": 4, "trail_big": 5, "act_mod": 0, "fo_split": False, "atp_bufs": 6,
       "act_last": True, "fp8_v": False, "ytc_act": True,
       "exp_pair": False, "psy_split": False, "fin01_inline": False,
       "defer_s0": 99, "pipe2": False, "atp2_bufs": 20, "pre_n": 0,
       "y2p_bufs": 2, "ytp_bufs": 2, "ytc_s0": 4, "boot2": True,
       "fo_psy_last": True, "dma_split_last": False, "pre_weave": False,
       "norm_merge": False, "ytc_final": False, "boot_tail": "b",
       "qkv_shift": 0,
       "qkv_quota": None}
F8 = mybir.dt.float8e4
DR = mybir.MatmulPerfMode.DoubleRow
OSCALE = 1.0 / 64.0  # wp x16, y x4 on device; undo in the psum drain


UNIT_LOG = []


def _mark(nc, label):
    if CFG.get("log_units"):
        UNIT_LOG.append((label, int(nc.next_id())))


def build_program(B=2, T=2048, C=1024):
    assert T % 512 == 0 and C % 128 == 0
    NCH = T // 512   # 512-token chunks per batch
    KT = C // 128    # contraction tiles for the qkv projection
    NTK = T // 128   # 128-key tiles per batch

    nc = bacc.Bacc("TRN2", target_bir_lowering=False, debug=False)
    xt = nc.dram_tensor("xt", [C, B * T], F16, kind="ExternalInput").ap()
    if CFG["fp8_v"]:
        xt8 = nc.dram_tensor("xt8", [C, B * T], F8, kind="ExternalInput").ap()
        wv8 = nc.dram_tensor("wv8", [64, KT * 2 * JW], F8,
                             kind="ExternalInput").ap()
    else:
        xt8 = wv8 = None
    # weights pre-packed on host: row p holds [kt, 128] contiguous
    if CFG["boot2"] and not CFG["fp8_v"] and not CFG["fp8_out"]:
        # bq (bitcast as 2 f16 cols) + per-kt interleaved wq|wk blocks:
        # one tensor so the startup loads it in two DMAs
        wqkb = nc.dram_tensor("wqkb", [128, 2 + KT * 2 * JW], F16,
                              kind="ExternalInput").ap()
        wq = wk = bq = None
    else:
        wqkb = None
        wq = nc.dram_tensor("wq", [128, KT * JW], F16, kind="ExternalInput").ap()
        wk = nc.dram_tensor("wk", [128, KT * JW], F16, kind="ExternalInput").ap()
        bq = nc.dram_tensor("bq", [JW, 1], F32, kind="ExternalInput").ap()
    wv = nc.dram_tensor("wv", [128, KT * JW], F16, kind="ExternalInput").ap()
    if CFG["fp8_out"]:
        # DoubleRow layout: dim d of the JW contraction lives at
        # (partition d % 64, k-tile d // 64); host packs wp to match.
        wp = nc.dram_tensor("wp", [64, 2 * C], F8, kind="ExternalInput").ap()
    else:
        wp = nc.dram_tensor("wp", [JW, C], F16, kind="ExternalInput").ap()
    out = nc.dram_tensor("out", [B * T, C], F16, kind="ExternalOutput").ap()
    out32 = None  # psum->dram direct ship is impossible: dma_start forbids PSUM src

    xt_r = xt.rearrange("(kt p) t -> p kt t", p=128)
    xt8_r = xt8.rearrange("(kt t p) tok -> p kt t tok", p=64, t=2) if xt8 is not None else None

    with tile.TileContext(nc) as tc:
        _build(tc, B, T, C, NCH, KT, NTK, xt_r, wq, wk, wv, bq, wp, out,
               xt8_r, wv8, wqkb, out32)
    nc.compile()
    return nc


def _build(tc, B, T, C, NCH, KT, NTK, xt_r, wq, wk, wv, bq, wp, out,
           xt8_r=None, wv8=None, wqkb=None, out32=None):
    nc = tc.nc
    from contextlib import ExitStack

    from concourse import library_config

    nc.gpsimd.load_library(library_config.attn)

    with ExitStack() as ctx:
        const = ctx.enter_context(tc.tile_pool(name="const", bufs=1))
        wpool = ctx.enter_context(tc.tile_pool(name="wpool", bufs=1))
        pbp = ctx.enter_context(tc.tile_pool(name="pbp", bufs=1))
        xtp = ctx.enter_context(tc.tile_pool(name="xtp", bufs=CFG.get("xtp_bufs", 2)))
        atp_bufs = (CFG["atp2_bufs"] if CFG["pipe2"]
                    else CFG["atp_bufs"] + CFG["pre_n"])
        atp = ctx.enter_context(tc.tile_pool(name="atp", bufs=atp_bufs))
        y2p = ctx.enter_context(tc.tile_pool(name="y2p", bufs=CFG["y2p_bufs"]))
        ytp = ctx.enter_context(tc.tile_pool(name="ytp", bufs=CFG["ytp_bufs"]))
        rcp = ctx.enter_context(tc.tile_pool(name="rcp", bufs=2))
        osp = ctx.enter_context(tc.tile_pool(name="osp", bufs=CFG.get("osp_bufs", 4)))
        psS = ctx.enter_context(tc.tile_pool(name="psS", bufs=2, space="PSUM"))
        psY = ctx.enter_context(tc.tile_pool(name="psY", bufs=1, space="PSUM"))
        psP = ctx.enter_context(tc.tile_pool(name="psP", bufs=2, space="PSUM"))

        # constants: transpose identity + upper-triangular causal mask (fp16)
        ident = const.tile([128, 128], R32)
        mask = const.tile([128, 128], F16)
        ident16 = const.tile([128, 128], F16)
        if wqkb is None:
            bq_sb = const.tile([JW, 1], F32)

        def emit_consts():
            nc.gpsimd.memset(ident[:].bitcast(mybir.dt.uint32), 0)
            nc.gpsimd.affine_select(
                out=ident[:], in_=ident[:],
                compare_op=mybir.AluOpType.not_equal, fill=1.0,
                base=0, pattern=[[-1, 128]], channel_multiplier=1,
            )
            # mask[p, c] = 1.0 if c >= p else 0  (valid: query c >= key p)
            nc.gpsimd.memset(mask[:].bitcast(U16), 15360)  # fp16 1.0
            nc.gpsimd.affine_select(
                out=mask[:], in_=mask[:],
                compare_op=mybir.AluOpType.is_ge, fill=0.0,
                base=0, pattern=[[1, 128]], channel_multiplier=-1,
            )
            nc.gpsimd.memset(ident16[:].bitcast(U16), 0)
            nc.gpsimd.affine_select(
                out=ident16[:], in_=ident16[:],
                compare_op=mybir.AluOpType.not_equal, fill=1.0,
                base=0, pattern=[[-1, 128]], channel_multiplier=1,
            )
            for _b in range(B):
                nc.gpsimd.memset(
                    vsbs[_b][:, :, :, HD : HD + 1].bitcast(U16), 15360)

        if wqkb is not None:
            wqkb_sb = wpool.tile([128, 2 + KT * 2 * JW], F16)
            bq_ref = wqkb_sb[:, 0:2].bitcast(F32)
            wqk_r = wqkb_sb[:, 2:].rearrange(
                "p (kt two j) -> p kt two j", kt=KT, two=2)

            def wq_kt(kt):
                return wqk_r[:, kt, 0]

            def wk_kt(kt):
                return wqk_r[:, kt, 1]
        else:
            wq_sb = wpool.tile([128, KT, JW], F16)
            wk_sb = wpool.tile([128, KT, JW], F16)
            bq_ref = None

            def wq_kt(kt):
                return wq_sb[:, kt]

            def wk_kt(kt):
                return wk_sb[:, kt]
        if CFG["fp8_v"]:
            wv_sb = wpool.tile([64, KT, 2, JW], F8)
        else:
            wv_sb = wpool.tile([128, KT, JW], F16)
        if CFG["fp8_out"]:
            wp_sb = wpool.tile([64, 2, C], F8)
        else:
            wp_sb = wpool.tile([JW, C], F16)

        # persistent per-batch tensors
        qTs, kTs, vsbs = {}, {}, {}
        for b in range(B):
            qT = pbp.tile([JW, T], F16, tag=f"qT{b}")
            kT = pbp.tile([JW, T], F16, tag=f"kT{b}")
            # v natural layout: [tok-in-tile, key tile, head, hd + ones col]
            vsb = pbp.tile([128, NTK, 2, HD + 1], F16, tag=f"vsb{b}")
            qTs[b], kTs[b], vsbs[b] = qT, kT, vsb

        def qkv_units(b, j, pre_xt=None):
            """qkv projection for 512-token chunk j of batch b, as ~1us
            emission units so it can weave into an attention tk-loop."""
            col0 = b * T + 512 * j
            st = {}

            def u_load_q03():
                _mark(nc, f'qkv{b}.{j}:q03')
                if pre_xt is not None:
                    xt_t = pre_xt
                else:
                    xt_t = xtp.tile([128, KT, 512], F16, tag="xt")
                    if j == 0:
                        for kk in range(0, KT, 2):
                            nc.sync.dma_start(xt_t[:, kk : kk + 2],
                                              xt_r[:, kk : kk + 2, ds(col0, 512)])
                    else:
                        nc.sync.dma_start(xt_t[:, 0:4],
                                          xt_r[:, 0:4, ds(col0, 512)])
                        nc.sync.dma_start(xt_t[:, 4:KT],
                                          xt_r[:, 4:KT, ds(col0, 512)])
                st["xt"] = xt_t
                if CFG["fp8_v"]:
                    xt8_t = xtp.tile([64, KT, 2, 512], F8, tag="xt8")
                    nc.sync.dma_start(xt8_t[:],
                                      xt8_r[:, :, :, ds(col0, 512)])
                    st["xt8"] = xt8_t
                psq = psP.tile([128, 512], F32, tag="p")
                st["psq"] = psq
                for kt in range(4):
                    nc.tensor.matmul(psq[:], wq_kt(kt), xt_t[:, kt],
                                     start=(kt == 0), stop=False)

            def u_q47():
                _mark(nc, f'qkv{b}.{j}:q47')
                xt_t, psq = st["xt"], st["psq"]
                for kt in range(4, KT):
                    nc.tensor.matmul(psq[:], wq_kt(kt), xt_t[:, kt],
                                     start=False, stop=(kt == KT - 1))
                bqv = bq_ref if bq_ref is not None else bq_sb[:]
                nc.vector.tensor_tensor(qTs[b][:, ts(j, 512)], psq[:],
                                        bqv.to_broadcast([JW, 512]), ADD)

            def u_k03():
                _mark(nc, f'qkv{b}.{j}:k03')
                psk = psP.tile([128, 512], F32, tag="p")
                st["psk"] = psk
                for kt in range(4):
                    nc.tensor.matmul(psk[:], wk_kt(kt), st["xt"][:, kt],
                                     start=(kt == 0), stop=False)

            def u_k47():
                _mark(nc, f'qkv{b}.{j}:k47')
                psk = st["psk"]
                for kt in range(4, KT):
                    nc.tensor.matmul(psk[:], wk_kt(kt), st["xt"][:, kt],
                                     start=False, stop=(kt == KT - 1))
                nc.vector.tensor_copy(kTs[b][:, ts(j, 512)], psk[:])

            def u_v01():
                _mark(nc, f'qkv{b}.{j}:v01')
                psv = psP.tile([128, 4, 128], F32, tag="p")
                st["psv"] = psv
                for t4 in range(2):
                    for kt in range(KT):
                        if CFG["fp8_v"]:
                            nc.tensor.matmul(psv[:, t4, :],
                                             st["xt8"][0:64, kt, :, ts(t4, 128)],
                                             wv_sb[0:64, kt, :, :],
                                             start=(t4 == 0 and kt == 0),
                                             stop=False, perf_mode=DR)
                        else:
                            nc.tensor.matmul(psv[:, t4, :],
                                             st["xt"][:, kt, ts(t4, 128)],
                                             wv_sb[:, kt],
                                             start=(t4 == 0 and kt == 0),
                                             stop=False)

            def u_v23():
                _mark(nc, f'qkv{b}.{j}:v23')
                psv = st["psv"]
                for t4 in range(2, 4):
                    for kt in range(KT):
                        if CFG["fp8_v"]:
                            nc.tensor.matmul(psv[:, t4, :],
                                             st["xt8"][0:64, kt, :, ts(t4, 128)],
                                             wv_sb[0:64, kt, :, :],
                                             start=False,
                                             stop=(t4 == 3 and kt == KT - 1),
                                             perf_mode=DR)
                        else:
                            nc.tensor.matmul(psv[:, t4, :],
                                             st["xt"][:, kt, ts(t4, 128)],
                                             wv_sb[:, kt],
                                             start=False,
                                             stop=(t4 == 3 and kt == KT - 1))
                if CFG["fp8_v"]:
                    nc.vector.tensor_scalar(
                        vsbs[b][:, ds(4 * j, 4), :, 0:HD],
                        psv[:].rearrange("p t4 (h d) -> p t4 h d", h=2),
                        1.0 / 16.0, None, MUL)
                else:
                    nc.vector.tensor_copy(
                        vsbs[b][:, ds(4 * j, 4), :, 0:HD],
                        psv[:].rearrange("p t4 (h d) -> p t4 h d", h=2),
                    )

            return [u_load_q03, u_q47, u_k03, u_k47, u_v01, u_v23]

        # ---- pipe2: scores+exp for chunk s+1 are emitted as filler of step
        # s (one step ahead of their attn@v), so no step ever waits on its
        # own exp and the final step has no ACT work at all. qkv runs two
        # steps ahead to feed the advanced scores.
        aT_store = {}

        def mk_s_units(b, j):
            """One scores+exp+mask unit per key tile of chunk (b, j)."""
            qT, kT = qTs[b], kTs[b]

            def mk(tk):
                def u():
                    _mark(nc, f'att{b}.{j}:s{tk}')
                    c0 = max(0, 128 * tk - 512 * j)
                    pss = psS.tile([128, 2, 512], F32, tag="s", name="pss")
                    for h in range(2):
                        nc.tensor.matmul(
                            pss[:, h, c0:512],
                            kT[ds(HD * h, HD), ts(tk, 128)],
                            qT[ds(HD * h, HD), ds(512 * j + c0, 512 - c0)],
                            start=True, stop=True,
                        )
                    aT = atp.tile([128, 2, 512], F16, tag="aT", name="aT")
                    aT_store[(b, j, tk)] = (aT, None)
                    nc.scalar.activation(aT[:, :, c0:512], pss[:, :, c0:512],
                                         EXP, scale=0.125)
                    if tk >= 4 * j:
                        d = tk - 4 * j
                        for h in range(2):
                            nc.vector.tensor_tensor(
                                aT[:, h, ts(d, 128)], aT[:, h, ts(d, 128)],
                                mask[:], MUL,
                            )
                return u

            return [mk(tk) for tk in range(4 * (j + 1))]

        def attn_step2(b, j, prev_norms, fills, final=False):
            """attn@v for chunk (b, j) (aT tiles precomputed last step),
            paced against `fills`. prev_norms run first: they read the
            previous chunk's psy banks, which this chunk's accumulation
            reuses."""
            vsb = vsbs[b]
            ntk = 4 * (j + 1)
            last01 = 4 * j + 1
            for u in prev_norms:
                u()
            psy01 = psY.tile([128, 2, 2, HD + 1], F32, tag="y01",
                             padded_shape=[128, 2, 2, 128], name="psy01")
            psy23 = psY.tile([128, 2, 2, HD + 1], F32, tag="y23",
                             padded_shape=[128, 2, 2, 128], name="psy23")
            st = {}

            def a_unit(tk):
                _mark(nc, f'att{b}.{j}:a{tk}')
                aT, _ = aT_store.pop((b, j, tk))
                for qq in range(4):
                    qg = 4 * j + qq
                    if qg < tk:
                        continue
                    for h in range(2):
                        psy, qi = (psy01, qq) if qq < 2 else (psy23, qq - 2)
                        nc.tensor.matmul(
                            psy[:, h, qi, :],
                            aT[:, h, ts(qq, 128)], vsb[:, tk, h, :],
                            start=(tk == 0 and qq in (0, 2) and h == 0),
                            stop=(h == 1 and ((qq == 1 and tk == last01)
                                              or (qq == 3 and tk == ntk - 1))),
                        )

            def fu_norm(p0):
                _mark(nc, f'att{b}.{j}:norm{p0}')
                if "y2" not in st:
                    st["y2"] = y2p.tile([128, 4, 2, HD], F16, tag="y2",
                                        name="y2")
                y2 = st["y2"]
                psy = psy01 if p0 == 0 else psy23
                for h in range(2):
                    rc = rcp.tile([128, 2, 1], F32, tag=f"rc{h}")
                    nc.vector.reciprocal(rc[:], psy[:, h, :, HD : HD + 1])
                    nc.vector.tensor_tensor(
                        y2[:, ds(p0, 2), h, :], psy[:, h, :, 0:HD],
                        rc[:].to_broadcast([128, 2, HD]), MUL,
                    )

            def fu_transp(p0):
                _mark(nc, f'att{b}.{j}:transp{p0}')
                if "yT2" not in st:
                    yT2 = ytp.tile([128, 4, 128], F16, tag="yT2", name="yT2")
                    st["yT2"] = yT2
                yT2 = st["yT2"]
                yT2p = psP.tile([128, 2, 128], F16, tag="p", name="yT2p",
                                padded_shape=[128, 2, 512])
                for iq, qq in enumerate((p0, p0 + 1)):
                    nc.tensor.matmul(yT2p[:, iq, :],
                                     st["y2"][:, qq, :, :], ident16[:],
                                     is_transpose=True,
                                     start=(iq == 0), stop=(iq == 1))
                if CFG.get("ytc_act"):
                    nc.scalar.activation(yT2[:, ds(p0, 2), :], yT2p[:],
                                         mybir.ActivationFunctionType.Copy)
                else:
                    nc.vector.tensor_copy(yT2[:, ds(p0, 2), :], yT2p[:])

            def fo(qq):
                _mark(nc, f'att{b}.{j}:fo{qq}')
                row0 = b * T + 512 * j + 128 * qq
                osb = osp.tile([128, C], F16, tag="osb", name="osb")
                for nh in range(2):
                    pso = psP.tile([128, 512], F32, tag="p", name="pso")
                    nc.tensor.matmul(pso[:], st["yT2"][:, qq, :],
                                     wp_sb[:, ts(nh, 512)],
                                     start=True, stop=True)
                    if nh == 1 and CFG.get("act_last") and b == B - 1 and j == NCH - 1:
                        nc.scalar.activation(
                            osb[:, ts(nh, 512)], pso[:],
                            mybir.ActivationFunctionType.Copy)
                    else:
                        nc.vector.tensor_copy(osb[:, ts(nh, 512)], pso[:])
                nc.sync.dma_start(out[ds(row0, 128), :], osb[:])

            inline01 = ([lambda: fu_norm(0), lambda: fu_transp(0),
                         lambda: fo(0), lambda: fo(1)] if final else [])
            if final:
                norms = [lambda: fu_norm(2)]
                rest = [lambda: fu_transp(2), lambda: fo(2), lambda: fo(3)]
            else:
                norms = [lambda: fu_norm(0), lambda: fu_norm(2)]
                rest = [lambda: fu_transp(0), lambda: fo(0), lambda: fo(1),
                        lambda: fu_transp(2), lambda: fo(2), lambda: fo(3)]
            pending = list(fills)
            total_fill = len(pending)
            emitted = 0
            for tk in range(ntk):
                a_unit(tk)
                if final and tk == last01:
                    for u in inline01:
                        u()
                target = -(-total_fill * (tk + 1) // ntk)  # ceil
                while emitted < target and pending:
                    pending.pop(0)()
                    emitted += 1
            for u in pending:
                u()
            return norms, rest

        def attn_step(b, j, extra_units, defer=False, final=False, s_idx=0):
            """One pipeline step: the attention tk-loop for chunk (b, j) with
            qkv units for the next chunk and this chunk's own normalize/
            transpose/output-projection units woven in as PE filler."""
            qT, kT, vsb = qTs[b], kTs[b], vsbs[b]
            ntk = 4 * (j + 1)
            last01 = 4 * j + 1  # last key tile contributing to q-tiles 0,1
            if CFG["psy_split"]:
                # q-tiles 0,1 and 2,3 in separate psum banks: the 0,1 group
                # stops at tk=last01 so its normalize/transpose/out-proj can
                # overlap the remaining key tiles' attn@v.
                psy01 = psY.tile([128, 2, 2, HD + 1], F32, tag="y01",
                                 padded_shape=[128, 2, 2, 128], name="psy01")
                psy23 = psY.tile([128, 2, 2, HD + 1], F32, tag="y23",
                                 padded_shape=[128, 2, 2, 128], name="psy23")
            else:
                psyA = psY.tile([128, 4, HD + 1], F32, tag="yA",
                                padded_shape=[128, 4, 128])
                psyB = psY.tile([128, 4, HD + 1], F32, tag="yB",
                                padded_shape=[128, 4, 128])
            st = {}

            aTs = {}

            def s_unit2(m):
                _mark(nc, f'att{b}.{j}:s2_{m}')
                """scores + exp + diagonal masks for key tiles 2m, 2m+1.
                One fp16-psum tile and ONE exp for the pair: the activation's
                ~370ns fixed access cost is paid once per two key tiles. For
                a diagonal pair, tk=2m+1's columns c0a:c0b hold exp of stale
                psum - never read (a_unit skips query blocks < key block)."""
                tka = 2 * m
                c0a = max(0, 128 * tka - 512 * j)
                pss = psS.tile([128, 2, 2, 512], F16, tag="s", name="pss")
                for i in range(2):
                    tk = tka + i
                    c0 = max(0, 128 * tk - 512 * j)
                    for h in range(2):
                        nc.tensor.matmul(
                            pss[:, i, h, c0:512],
                            kT[ds(HD * h, HD), ts(tk, 128)],
                            qT[ds(HD * h, HD), ds(512 * j + c0, 512 - c0)],
                            start=(h == 0), stop=(h == 1),
                        )
                aT = atp.tile([128, 2, 2, 512], F16, tag="aT", name="aT")
                aT_store[(b, j, tka)] = (aT, 0)
                aT_store[(b, j, tka + 1)] = (aT, 1)
                nc.scalar.activation(aT[:, :, :, c0a:512],
                                     pss[:, :, :, c0a:512], EXP, scale=0.125)
                for i in range(2):
                    tk = tka + i
                    if tk >= 4 * j:
                        d = tk - 4 * j  # diagonal q-tile index within chunk
                        for h in range(2):
                            nc.vector.tensor_tensor(
                                aT[:, i, h, ts(d, 128)],
                                aT[:, i, h, ts(d, 128)], mask[:], MUL,
                            )

            def s_unit(tk):
                """scores + exp + diagonal mask for key tile tk."""
                if (b, j, tk) in aT_store:
                    return  # precomputed in an earlier step
                _mark(nc, f'att{b}.{j}:s{tk}')
                c0 = max(0, 128 * tk - 512 * j)
                pss = psS.tile([128, 2, 512], F32, tag="s")
                for h in range(2):
                    nc.tensor.matmul(
                        pss[:, h, c0:512],
                        kT[ds(HD * h, HD), ts(tk, 128)],
                        qT[ds(HD * h, HD), ds(512 * j + c0, 512 - c0)],
                        start=True, stop=True,
                    )
                aT = atp.tile([128, 2, 512], F16, tag="aT")
                aT_store[(b, j, tk)] = (aT, None)
                nc.scalar.activation(aT[:, :, c0:512], pss[:, :, c0:512],
                                     EXP, scale=0.125)
                if tk >= 4 * j:
                    d = tk - 4 * j  # diagonal q-tile index within chunk
                    # DVE, not gpsimd: the Pool Q7 launch latency and its
                    # sem-blocked in-order queue stall the diagonal attn@v
                    for h in range(2):
                        nc.vector.tensor_tensor(
                            aT[:, h, ts(d, 128)], aT[:, h, ts(d, 128)],
                            mask[:], MUL,
                        )

            def a_unit(tk):
                _mark(nc, f'att{b}.{j}:a{tk}')
                """attn@v accumulation for key tile tk (runs one iteration
                behind s_unit so the exp has left the ACT queue)."""
                aT, i = aT_store.pop((b, j, tk))
                for qq in CFG.get("qq_order", (0, 1, 2, 3)):
                    qg = 4 * j + qq  # global q-tile index
                    if qg < tk:
                        continue
                    for h in range(2):
                        ab = (aT[:, i, h, ts(qq, 128)] if i is not None
                              else aT[:, h, ts(qq, 128)])
                        if CFG["psy_split"]:
                            psy, qi = (psy01, qq) if qq < 2 else (psy23, qq - 2)
                            nc.tensor.matmul(
                                psy[:, h, qi, :], ab, vsb[:, tk, h, :],
                                start=(tk == 0 and qq in (0, 2) and h == 0),
                                stop=(h == 1 and ((qq == 1 and tk == last01)
                                                  or (qq == 3 and tk == ntk - 1))),
                            )
                        else:
                            psy = psyA if h == 0 else psyB
                            nc.tensor.matmul(
                                psy[:, qq, :], ab, vsb[:, tk, h, :],
                                start=(tk == 0 and qq == 0),
                                stop=(tk == ntk - 1 and qq == 3),
                            )

            rcs = {}

            def fu_normA(p0):
                """half of the merged normalize: p0=0 also does both
                reciprocals; emitted interleaved with the transposes so the
                DVE in-order queue releases transp0's copy before the q23
                multiplies run (the ACT fo-copy chain starts earlier)."""
                _mark(nc, f'att{b}.{j}:normA{p0}')
                if "y2" not in st:
                    st["y2"] = y2p.tile([128, 4, 2, HD], F16, tag="y2", name="y2")
                y2 = st["y2"]
                if p0 == 0:
                    for h, psy in ((0, psyA), (1, psyB)):
                        rc = rcp.tile([128, 4, 1], F32, tag=f"rca{h}")
                        nc.vector.reciprocal(rc[:], psy[:, :, HD : HD + 1])
                        rcs[h] = rc
                for h, psy in ((0, psyA), (1, psyB)):
                    nc.vector.tensor_tensor(
                        y2[:, ds(p0, 2), h, :], psy[:, ds(p0, 2), 0:HD],
                        rcs[h][:, ds(p0, 2)].to_broadcast([128, 2, HD]),
                        MUL,
                    )

            def fu_norm_all():
                fu_normA(0)
                fu_normA(2)

            def fu_norm(p0):
                _mark(nc, f'att{b}.{j}:norm{p0}')
                """normalize q-tiles p0, p0+1 (attn@v chains stopped): DVE."""
                if "y2" not in st:
                    st["y2"] = y2p.tile([128, 4, 2, HD], F16, tag="y2", name="y2")
                y2 = st["y2"]
                if CFG["psy_split"]:
                    psy = psy01 if p0 == 0 else psy23
                    for h in range(2):
                        rc = rcp.tile([128, 2, 1], F32, tag=f"rc{h}")
                        nc.vector.reciprocal(rc[:], psy[:, h, :, HD : HD + 1])
                        nc.vector.tensor_tensor(
                            y2[:, ds(p0, 2), h, :], psy[:, h, :, 0:HD],
                            rc[:].to_broadcast([128, 2, HD]), MUL,
                        )
                    return
                for h, psy in ((0, psyA), (1, psyB)):
                    rc = rcp.tile([128, 2, 1], F32, tag=f"rc{h}")
                    nc.vector.reciprocal(rc[:], psy[:, ds(p0, 2), HD : HD + 1])
                    nc.vector.tensor_tensor(
                        y2[:, ds(p0, 2), h, :], psy[:, ds(p0, 2), 0:HD],
                        rc[:].to_broadcast([128, 2, HD]), MUL,
                    )

            def fu_transp(p0):
                _mark(nc, f'att{b}.{j}:transp{p0}')
                """transpose q-tiles p0, p0+1 to yT layout. Non-final chunks
                use the DMA xbar transpose (14ns/16x128 tile): no PE matmuls,
                no psum round-trip, no ACT/DVE drain copy - the ~2.5us DMA
                latency is hidden because the finish weaves into the next
                step. The final chunk keeps the low-latency PE path."""
                if (CFG.get("transp_dma") and not final
                        and s_idx < CFG.get("transp_dma_s1", 99)
                        and not CFG["fp8_out"]):
                    if "yT2" not in st:
                        st["yT2"] = ytp.tile([128, 4, 128], F16, tag="yT2",
                                             name="yT2")
                    for qq in (p0, p0 + 1):
                        nc.sync.dma_start_transpose(
                            st["yT2"][:, qq, :], st["y2"][:, qq, :, :])
                    return
                if CFG["fp8_out"]:
                    # split transposes land both JW halves on partitions
                    # 0..63, giving the [64, ktile, tok] DoubleRow layout
                    if "yT2" not in st:
                        st["yT2"] = ytp.tile([64, 4, 2, 128], F8, tag="yT2", name="yT28")
                    yT28 = st["yT2"]
                    yT2p8 = psP.tile([64, 2, 2, 128], R32, tag="p", name="yT2p8")
                    for iq, qq in enumerate((p0, p0 + 1)):
                        for t in range(2):
                            nc.tensor.matmul(
                                yT2p8[0:64, iq, t, :],
                                st["y2"][:, qq, t, :], ident[:],
                                is_transpose=True,
                                start=(iq == 0 and t == 0),
                                stop=(iq == 1 and t == 1))
                    nc.vector.tensor_scalar(
                        yT28[0:64, ds(p0, 2), :, :], yT2p8[0:64],
                        4.0, None, MUL)
                    return
                if "yT2" not in st:
                    yT2 = ytp.tile([128, 4, 128], F16, tag="yT2")
                    st["yT2"] = yT2
                yT2 = st["yT2"]
                yT2p = psP.tile([128, 2, 128], F16, tag="p", name="yT2p",
                                padded_shape=[128, 2, 512])
                for iq, qq in enumerate((p0, p0 + 1)):
                    nc.tensor.matmul(yT2p[:, iq, :],
                                     st["y2"][:, qq, :, :], ident16[:],
                                     is_transpose=True,
                                     start=(iq == 0), stop=(iq == 1))
                use_act = CFG.get("ytc_act") and (
                    s_idx < CFG["ytc_s0"]
                    or (final and CFG.get("ytc_final", True)))
                if final and CFG.get("ytc_split_final", True) and p0 == 0:
                    use_act = False  # DVE is free right after the norms
                if use_act:
                    nc.scalar.activation(yT2[:, ds(p0, 2), :], yT2p[:],
                                         mybir.ActivationFunctionType.Copy)
                else:
                    nc.vector.tensor_copy(yT2[:, ds(p0, 2), :], yT2p[:])

            def fo(qq, nhs=(0, 1)):
                _mark(nc, f'att{b}.{j}:fo{qq}')
                row0 = b * T + 512 * j + 128 * qq
                merge = final and CFG.get("fo_merge_last", False)
                if merge:
                    pair = qq // 2
                    if qq % 2 == 0 and 0 in nhs:
                        st[f"osb2_{pair}"] = osp.tile(
                            [128, 2, C], F16, tag="osb2", name="osb2")
                    st[f"osb{qq}"] = st[f"osb2_{pair}"][:, qq % 2]
                elif 0 in nhs:
                    st[f"osb{qq}"] = osp.tile([128, C], F16, tag="osb",
                                              name="osb")
                osb = st[f"osb{qq}"]
                # final chunk: after the norms, psyA/psyB banks are dead -
                # use them as two extra pso buffers so the four out-proj
                # matmuls stream without waiting on the psP drain rotation.
                use_yab = (CFG.get("fo_psy_last") and not CFG["psy_split"]
                           and final and qq in (1, 3))
                use_s = (CFG.get("fo_pss_last", True) and final
                         and qq in CFG.get("fo_pss_qq", (0,)))
                raw32 = (CFG.get("fo3_raw32", False) and final and qq == 3
                         and out32 is not None and not CFG["fp8_out"])
                for nh in nhs:
                    if use_yab:
                        pso = psY.tile([128, 512], F32,
                                       tag=("yA" if nh == 0 else "yB"),
                                       name="psoY")
                    elif use_s:
                        pso = psS.tile([128, 512], F32, tag="s", name="psoS")
                    else:
                        pso = psP.tile([128, 512], F32, tag="p")
                    if CFG["fp8_out"]:
                        nc.tensor.matmul(pso[:], st["yT2"][0:64, qq, :, :],
                                         wp_sb[0:64, :, ts(nh, 512)],
                                         start=True, stop=True, perf_mode=DR)
                    else:
                        nc.tensor.matmul(pso[:], st["yT2"][:, qq, :],
                                         wp_sb[:, ts(nh, 512)],
                                         start=True, stop=True)
                    if raw32:
                        # last q-tile: psum straight to dram in fp32, skipping
                        # the drain-copy hop that gates program end (the host
                        # sums partials in fp32 anyway)
                        nc.sync.dma_start(out32[:, ts(nh, 512)], pso[:])
                        continue
    
                    if final and CFG.get("act_alt_last", False):
                        if (qq + nh) % 2 == 0:
                            nc.scalar.activation(
                                osb[:, ts(nh, 512)], pso[:],
                                mybir.ActivationFunctionType.Copy)
                        else:
                            nc.vector.tensor_copy(osb[:, ts(nh, 512)], pso[:])
                        continue
                    if nh == 1 and ((CFG["act_mod"] and (j + qq) % CFG["act_mod"] == 0) or (CFG.get("act_last") and final) or (CFG.get("act_early") and j <= 0)):
                        nc.scalar.activation(
                            osb[:, ts(nh, 512)], pso[:],
                            mybir.ActivationFunctionType.Copy,
                            scale=OSCALE if CFG["fp8_out"] else 1.0,
                        )
                    elif CFG["fp8_out"]:
                        nc.vector.tensor_scalar(
                            osb[:, ts(nh, 512)], pso[:], OSCALE, None, MUL)
                    else:
                        nc.vector.tensor_copy(osb[:, ts(nh, 512)], pso[:])
                    if (CFG.get("dma_split_last") and b == B - 1
                            and j == NCH - 1):
                        nc.sync.dma_start(
                            out[ds(row0, 128), ts(nh, 512)],
                            osb[:, ts(nh, 512)])
                if raw32:
                    return
                if 1 in nhs and merge:
                    if qq % 2 == 1:
                        r0 = (b * T + 512 * j) // 128 + qq - 1
                        out_r = out.rearrange("(r p) c -> p r c", p=128)
                        nc.sync.dma_start(out_r[:, r0 : r0 + 2, :],
                                          st[f"osb2_{qq // 2}"][:])
                elif (1 in nhs and final and qq == 3
                      and CFG.get("dma_split_fo3", False)):
                    # the very last transfer gates program end: halve it so
                    # the nh0 half ships while nh1 still drains
                    nc.sync.dma_start(out[ds(row0, 128), ts(0, 512)],
                                      osb[:, ts(0, 512)])
                    nc.sync.dma_start(out[ds(row0, 128), ts(1, 512)],
                                      osb[:, ts(1, 512)])
                elif 1 in nhs and not (CFG.get("dma_split_last") and final):
                    nc.sync.dma_start(out[ds(row0, 128), :], osb[:])

            # the psum zero-region rule forbids reading psy mid-group, so
            # finish units run after their psum group stops. With psy_split,
            # the q-tiles-0,1 group stops at tk=last01: its normalize runs
            # inline right after (freeing the bank early), and for the final
            # chunk the whole 0,1 finish chain runs inline so the output
            # drains/DMAs overlap the tail key tiles' exp-bound attn@v.
            # `defer` steps push their transpose/out-proj PE work into the
            # final step, where PE otherwise idles behind ACT.
            finishA = [lambda: fu_transp(0), lambda: fo(0), lambda: fo(1)]
            if (final and CFG.get("norm_split_final", True)
                    and not CFG["psy_split"]):
                finishB = [lambda: fu_normA(2), lambda: fu_transp(2),
                           lambda: fo(2), lambda: fo(3)]
                norm01 = [lambda: fu_normA(0), lambda: fu_transp(0),
                          lambda: fo(0), lambda: fo(1)]
                finishA = []
            elif ((CFG.get("norm_merge") or (final and CFG.get("norm_merge_last", True)))
                    and not CFG["psy_split"]):
                finishB = [lambda: fu_transp(2), lambda: fo(2), lambda: fo(3)]
                norm01 = [fu_norm_all]
            else:
                finishB = [lambda: fu_norm(2), lambda: fu_transp(2),
                           lambda: fo(2), lambda: fo(3)]
                norm01 = [lambda: fu_norm(0)]
            # inline norm01 mid-step would park a sem-blocked op at the head
            # of DVE's in-order queue and stall the woven qkv drains behind
            # it - only the final step (no downstream DVE consumers) inlines.
            inline01 = []
            if CFG["psy_split"] and final and CFG["fin01_inline"]:
                inline01 = norm01 + finishA
                carry, deferred = finishB, []
            elif CFG["psy_split"] and defer:
                carry, deferred = norm01 + [finishB[0]], finishA + finishB[1:]
            else:
                carry, deferred = norm01 + finishA + finishB, []
                if (CFG.get("transp_dma") and not final
                        and s_idx < CFG.get("transp_dma_s1", 99)):
                    # DMA transposes need their ~2.9us latency hidden:
                    # issue both early, draw the fo units late
                    carry = (norm01
                             + [lambda: fu_transp(0), lambda: fu_norm(2),
                                lambda: fu_transp(2), lambda: fo(0),
                                lambda: fo(1), lambda: fo(2), lambda: fo(3)])
            pending = list(extra_units)
            total_fill = len(pending)
            emitted = 0
            TRAIL = CFG["trail"] if ntk > 4 else CFG.get("trail_small", 1)
            if CFG.get("trail_big") and ntk >= 12:
                TRAIL = CFG["trail_big"]

            def emit_a(tk):
                a_unit(tk)
                if tk == last01:
                    for u in inline01:
                        u()

            for tk in range(ntk):
                if CFG["exp_pair"]:
                    if tk % 2 == 0:
                        s_unit2(tk // 2)
                else:
                    s_unit(tk)
                if final and CFG.get("fill_pow2_final"):
                    frac = float(tk + 1) / ntk
                    target = int(total_fill * frac * frac + 0.999)
                else:
                    target = -(-total_fill * (tk + 1) // ntk)  # ceil
                nb = CFG.get("fill_before_a", 1)
                while nb > 0 and emitted < target and pending:
                    pending.pop(0)()
                    emitted += 1
                    nb -= 1
                if tk >= TRAIL:
                    emit_a(tk - TRAIL)
                while emitted < target and pending:
                    pending.pop(0)()
                    emitted += 1
            for tk in range(ntk - TRAIL, ntk):
                emit_a(tk)
                if pending:
                    pending.pop(0)()
            for u in pending:
                u()
            return carry, deferred

        # ---- startup: stage DMAs so the first q matmuls start early ----
        if not (CFG["boot2"] and not CFG["fp8_v"] and not CFG["fp8_out"]):
            emit_consts()
        wq_r2 = wq.rearrange("p (kt j) -> p kt j", kt=KT) if wq is not None else None
        xt00 = xtp.tile([128, KT, 512], F16, tag="xt")
        if CFG["boot2"] and not CFG["fp8_v"] and not CFG["fp8_out"]:
            # kt-pair-major startup: q/k of chunk (0,0) consume each 256KB
            # xt slice as it lands, so PE food arrives per-DMA instead of
            # per-projection. The DMA stream is emitted in consumption
            # order (the single HWDGE serializes descriptor generation).
            xt10 = xtp.tile([128, KT, 512], F16, tag="xt", name="xt10")
            W0 = 2 + 2 * 2 * JW  # bq + kt0,kt1 interleaved blocks
            W1 = 2 + 2 * JW      # bq + kt0 block only
            bf = CFG.get("boot_first", "wq")
            if bf == "xt":
                nc.sync.dma_start(xt00[:, 0:2], xt_r[:, 0:2, ds(0, 512)])
                nc.sync.dma_start(wqkb_sb[:, 0:W0], wqkb[:, 0:W0])
            elif bf == "fine":
                nc.sync.dma_start(wqkb_sb[:, 0:W1], wqkb[:, 0:W1])
                nc.sync.dma_start(xt00[:, 0:1], xt_r[:, 0:1, ds(0, 512)])
                nc.sync.dma_start(wqkb_sb[:, W1:W0], wqkb[:, W1:W0])
                nc.sync.dma_start(xt00[:, 1:2], xt_r[:, 1:2, ds(0, 512)])
            else:
                nc.sync.dma_start(wqkb_sb[:, 0:W0], wqkb[:, 0:W0])
                nc.sync.dma_start(xt00[:, 0:2], xt_r[:, 0:2, ds(0, 512)])
            nc.sync.dma_start(wqkb_sb[:, W0:], wqkb[:, W0:])
            nc.sync.dma_start(xt00[:, 2:4], xt_r[:, 2:4, ds(0, 512)])
            tail_order = CFG.get("boot_tail", "a")
            if tail_order == "a":
                nc.sync.dma_start(xt00[:, 4:6], xt_r[:, 4:6, ds(0, 512)])
                nc.sync.dma_start(xt00[:, 6:KT], xt_r[:, 6:KT, ds(0, 512)])
                nc.sync.dma_start(wv_sb[:],
                                  wv.rearrange("p (kt j) -> p kt j", kt=KT))
                nc.sync.dma_start(xt10[:, 0:4], xt_r[:, 0:4, ds(T, 512)])
                nc.sync.dma_start(xt10[:, 4:KT], xt_r[:, 4:KT, ds(T, 512)])
                nc.sync.dma_start(wp_sb[:], wp)
            elif tail_order == "b":  # xt10a before wv
                wv_r = wv.rearrange("p (kt j) -> p kt j", kt=KT)
                nc.sync.dma_start(xt00[:, 4:6], xt_r[:, 4:6, ds(0, 512)])
                nc.sync.dma_start(xt00[:, 6:KT], xt_r[:, 6:KT, ds(0, 512)])
                if CFG.get("v0_kt_outer"):
                    nc.sync.dma_start(wv_sb[:, 0:2], wv_r[:, 0:2])
                    nc.sync.dma_start(xt10[:, 0:4], xt_r[:, 0:4, ds(T, 512)])
                    nc.sync.dma_start(wv_sb[:, 2:KT], wv_r[:, 2:KT])
                else:
                    nc.sync.dma_start(xt10[:, 0:4], xt_r[:, 0:4, ds(T, 512)])
                    nc.sync.dma_start(wv_sb[:], wv_r)
                nc.sync.dma_start(xt10[:, 4:KT], xt_r[:, 4:KT, ds(T, 512)])
                nc.sync.dma_start(wp_sb[:], wp)
            else:  # c: xt00 fully before weights tail
                nc.sync.dma_start(xt00[:, 4:KT], xt_r[:, 4:KT, ds(0, 512)])
                nc.sync.dma_start(wv_sb[:],
                                  wv.rearrange("p (kt j) -> p kt j", kt=KT))
                nc.sync.dma_start(xt10[:, 0:4], xt_r[:, 0:4, ds(T, 512)])
                nc.sync.dma_start(xt10[:, 4:KT], xt_r[:, 4:KT, ds(T, 512)])
                nc.sync.dma_start(wp_sb[:], wp)
            emit_consts()
            psq0 = psP.tile([128, 512], F32, tag="p", name="psq0")
            psk0 = psP.tile([128, 512], F32, tag="p", name="psk0")
            for kp in range(KT // 2):
                for kt in (2 * kp, 2 * kp + 1):
                    nc.tensor.matmul(psq0[:], wq_kt(kt), xt00[:, kt],
                                     start=(kt == 0), stop=(kt == KT - 1))
                for kt in (2 * kp, 2 * kp + 1):
                    nc.tensor.matmul(psk0[:], wk_kt(kt), xt00[:, kt],
                                     start=(kt == 0), stop=(kt == KT - 1))
            bqv0 = bq_ref if bq_ref is not None else bq_sb[:]
            nc.vector.tensor_tensor(qTs[0][:, ts(0, 512)], psq0[:],
                                    bqv0.to_broadcast([JW, 512]), ADD)
            nc.vector.tensor_copy(kTs[0][:, ts(0, 512)], psk0[:])
            # scores+exp for chunk (0,0) here: the PE wait-queue lets them
            # overtake the DMA-blocked v/qkv(1,0) matmuls, so the ACT exp
            # stream starts ~3us earlier in an otherwise idle window
            for u in mk_s_units(0, 0)[:CFG.get("pre00", 0)]:
                u()
            psv0 = psP.tile([128, 4, 128], F32, tag="p", name="psv0")
            if CFG.get("v0_kt_outer"):
                for kt in range(KT):
                    for t4 in range(4):
                        nc.tensor.matmul(psv0[:, t4, :],
                                         xt00[:, kt, ts(t4, 128)],
                                         wv_sb[:, kt],
                                         start=(kt == 0 and t4 == 0),
                                         stop=(kt == KT - 1 and t4 == 3))
            else:
                for t4 in range(4):
                    for kt in range(KT):
                        nc.tensor.matmul(psv0[:, t4, :],
                                         xt00[:, kt, ts(t4, 128)],
                                         wv_sb[:, kt],
                                         start=(t4 == 0 and kt == 0),
                                         stop=(t4 == 3 and kt == KT - 1))
            nc.vector.tensor_copy(
                vsbs[0][:, ds(0, 4), :, 0:HD],
                psv0[:].rearrange("p t4 (h d) -> p t4 h d", h=2))
            u0 = qkv_units(1, 0, pre_xt=xt10)
            for u in u0:
                u()
            boot2_done = True
        else:
            boot2_done = False
            nc.sync.dma_start(wq_sb[:, 0:2], wq_r2[:, 0:2])
            nc.sync.dma_start(xt00[:, 0:2], xt_r[:, 0:2, ds(0, 512)])
            nc.sync.dma_start(wq_sb[:, 2:4], wq_r2[:, 2:4])
            nc.sync.dma_start(xt00[:, 2:4], xt_r[:, 2:4, ds(0, 512)])
            nc.sync.dma_start(wq_sb[:, 4:KT], wq_r2[:, 4:KT])
            nc.sync.dma_start(xt00[:, 4:6], xt_r[:, 4:6, ds(0, 512)])
            nc.sync.dma_start(bq_sb[:], bq)
            nc.sync.dma_start(xt00[:, 6:KT], xt_r[:, 6:KT, ds(0, 512)])
            nc.sync.dma_start(wk_sb[:], wk.rearrange("p (kt j) -> p kt j", kt=KT))
            u0 = qkv_units(0, 0, pre_xt=xt00)
            u0[0]()  # first q matmuls
            u0[1]()
            if CFG["fp8_v"]:
                nc.sync.dma_start(wv_sb[:],
                                  wv8.rearrange("p (kt t j) -> p kt t j",
                                                kt=KT, t=2))
            else:
                nc.sync.dma_start(wv_sb[:],
                                  wv.rearrange("p (kt j) -> p kt j", kt=KT))
            u0[2]()
            u0[3]()
            if CFG["fp8_out"]:
                nc.sync.dma_start(wp_sb[:], wp.rearrange("p (t c) -> p t c", t=2))
            else:
                nc.sync.dma_start(wp_sb[:], wp)
            u0[4]()
            u0[5]()
        steps = [(b, j) for j in range(NCH) for b in range(B)]
        if CFG["pipe2"]:
            # startup tail: qkv(1,0) interleaved with scores(0,0)
            q10 = qkv_units(1, 0)
            s00 = mk_s_units(0, 0)
            seq = []
            for i in range(max(len(q10), len(s00))):
                if i < len(q10):
                    seq.append(q10[i])
                if i < len(s00):
                    seq.append(s00[i])
            for u in seq:
                u()
            norms, rest = [], []
            for s, (b, j) in enumerate(steps):
                final = s == len(steps) - 1
                s_next = mk_s_units(*steps[s + 1]) if s + 1 < len(steps) else []
                qk = qkv_units(*steps[s + 2]) if s + 2 < len(steps) else []
                inter = []
                for i in range(max(len(s_next), len(qk))):
                    if i < len(s_next):
                        inter.append(s_next[i])
                    if i < len(qk):
                        inter.append(qk[i])
                fills = rest + inter
                norms, rest = attn_step2(b, j, norms, fills, final=final)
            for u in norms + rest:
                u()
            return
        if not boot2_done:
            for u in qkv_units(1, 0):
                u()
        # just-in-time weave: qkv for step s+1's chunk fills step s, so the
        # tail steps (which have no qkv of their own left) stay PE-fed
        carried = []
        deferred_pool = []
        S0 = CFG["defer_s0"]
        # step_order: attention chunks may run in any order that satisfies
        # "attn(b,j) after qkv(b, j' <= j)"; ending on a SHALLOW chunk keeps
        # the final (fill-less) step's exp load small. qkv_order[s] names the
        # chunk whose qkv weaves into step s.
        if CFG.get("step_order"):
            steps = [tuple(x) for x in CFG["step_order"]]
            qkv_order = [tuple(x) for x in CFG["qkv_order"]]
        else:
            qkv_order = None
        # qkv fill queue: all remaining chunks' qkv units in step order,
        # drawn per-step by quota so the exp-latency-bound early steps get
        # PE fill too (their own chunks are too shallow to hide exp).
        quota = CFG.get("qkv_quota")
        if quota:
            qkv_queue = []
            for (qb, qj) in steps:
                if qj >= 1:
                    qkv_queue += qkv_units(qb, qj)
        qkv_cache = {}
        for s, (b, j) in enumerate(steps):
            final = s == len(steps) - 1
            if qkv_order is not None:
                qk = (qkv_units(*qkv_order[s])
                      if s < len(qkv_order) else [])
            elif quota:
                take = min(quota[s] if s < len(quota) else 99, len(qkv_queue))
                qk = qkv_queue[:take]
                del qkv_queue[:take]
            elif CFG["weave"] == "jit":
                sh = CFG.get("qkv_shift", 0)
                if sh:
                    # split each chunk's 6 qkv units sh/6-sh across two steps
                    qk = []
                    if s + 1 < len(steps) and steps[s + 1][1] >= 1:
                        qk += qkv_cache.setdefault(
                            steps[s + 1], qkv_units(*steps[s + 1]))[sh:]
                    if s + 2 < len(steps) and steps[s + 2][1] >= 1:
                        qk += qkv_cache.setdefault(
                            steps[s + 2], qkv_units(*steps[s + 2]))[:sh]
                elif s + 1 < len(steps) and steps[s + 1][1] >= 1:
                    qk = qkv_units(*steps[s + 1])
                else:
                    qk = []
            else:
                qk = qkv_units(b, j + 1) if j + 1 < NCH else []
            if final:
                qk = qk + deferred_pool
                deferred_pool = []
            # bucket brigade: precompute the next chunk's first pre_n
            # scores+exp during this step, cascading exp load away from the
            # ACT-bound final steps. Emitted after this step's qkv units
            # (which write the qT/kT they read).
            pre = []
            if CFG["pre_n"] > 0 and s + 1 < len(steps) and steps[s + 1][1] >= 1:
                pre = mk_s_units(*steps[s + 1])[:CFG["pre_n"]]
            # interleave the previous step's finish units with qkv units
            ilv = CFG.get("ilv", 0)
            if ilv == 1:
                extra = qk + carried
            elif ilv == 2:
                extra = carried + qk
            elif ilv == 3:  # qkv leads the alternation
                extra = []
                for i in range(max(len(carried), len(qk))):
                    if i < len(qk):
                        extra.append(qk[i])
                    if i < len(carried):
                        extra.append(carried[i])
            else:
                extra = []
                for i in range(max(len(carried), len(qk))):
                    if i < len(carried):
                        extra.append(carried[i])
                    if i < len(qk):
                        extra.append(qk[i])
            if CFG.get("pre_weave"):
                ex2 = []
                for i, u in enumerate(extra):
                    ex2.append(u)
                    if i >= 2 and i - 2 < len(pre):
                        ex2.append(pre[i - 2])
                extra = ex2 + pre[max(0, len(extra) - 2):]
            else:
                extra = extra + pre
            carried, dfr = attn_step(b, j, extra,
                                     defer=(S0 <= s < len(steps) - 1),
                                     final=final, s_idx=s)
            deferred_pool += dfr
        # tail: run normalize+transpose first so the four output
        # projections stream back-to-back with their DVE/ACT drains parallel
        if CFG.get("norm_split_final", True) and not CFG["psy_split"]:
            # pre-ordered: [nA0, t0, nA2, t2, fo0, fo1, fo2, fo3]
            carried = [carried[i] for i in (0, 1, 4, 5, 2, 3, 6, 7)]
        elif len(carried) == 8:
            carried = [carried[i] for i in (0, 1, 4, 5, 2, 3, 6, 7)]
        elif len(carried) == 7:  # norm_merge: [nA, t0, fo0, fo1, t2, fo2, fo3]
            carried = [carried[i] for i in (0, 1, 4, 2, 3, 5, 6)]
        for u in carried:
            u()


def _pack_wv8(wv):
    # [C, 128] -> [64, KT, 2, 128]: contraction dim c = kt*128 + t*64 + p
    import ml_dtypes
    KT = wv.shape[0] // 128
    w = (16.0 * wv).reshape(KT, 2, 64, 128).transpose(2, 0, 1, 3)
    return np.ascontiguousarray(w.reshape(64, -1).astype(ml_dtypes.float8_e4m3))


def _pack_wp(wp):
    if CFG["fp8_out"]:
        import ml_dtypes
        return np.ascontiguousarray(
            (16.0 * wp.reshape(2, 64, -1).transpose(1, 0, 2)
             .reshape(64, -1)).astype(ml_dtypes.float8_e4m3)
        )
    return np.ascontiguousarray(wp.astype(np.float16))


def make_in_maps(x, w_attn, b_attn, w_proj):
    B, T, C = x.shape
    KT = C // 128
    x = np.asarray(x, np.float32)
    w_attn = np.asarray(w_attn, np.float32)
    b_attn = np.asarray(b_attn, np.float32)
    w_proj = np.asarray(w_proj, np.float32)
    xt = np.ascontiguousarray(
        x.transpose(2, 0, 1).reshape(C, B * T).astype(np.float16)
    )
    if CFG["fp8_v"]:
        import ml_dtypes
        xt8 = np.ascontiguousarray(
            x.transpose(2, 0, 1).reshape(C, B * T).astype(ml_dtypes.float8_e4m3)
        )

    def pack(w):  # [C, 128] -> [128, KT*128], row p = [kt, j] contiguous
        return np.ascontiguousarray(
            w.reshape(KT, 128, 128).transpose(1, 0, 2).reshape(128, KT * 128)
        ).astype(np.float16)

    packed = CFG["boot2"] and not CFG["fp8_v"] and not CFG["fp8_out"]

    def pack_qkb(wqc, wkc, bqc):
        # [128, 2 + KT*2*128]: row p = [bq(f32 as 2xf16) | kt: wq_kt | wk_kt]
        wqk = np.stack(
            [wqc.reshape(KT, 128, 128), wkc.reshape(KT, 128, 128)], axis=1
        ).transpose(2, 0, 1, 3).reshape(128, KT * 2 * 128).astype(np.float16)
        head = np.ascontiguousarray(
            bqc.astype(np.float32).reshape(128, 1)).view(np.float16)
        return np.ascontiguousarray(np.concatenate([head, wqk], axis=1))

    in_maps = []
    for i in range(N_CORES):
        h0 = i * JW
        extra8 = (
            {"xt8": xt8,
             "wv8": _pack_wv8(w_attn[:, 2 * C + h0 : 2 * C + h0 + JW])}
            if CFG["fp8_v"] else {}
        )
        if packed:
            wqk_entries = {
                "wqkb": pack_qkb(w_attn[:, h0 : h0 + JW],
                                 w_attn[:, C + h0 : C + h0 + JW],
                                 b_attn[h0 : h0 + JW]),
            }
        else:
            wqk_entries = {
                "wq": pack(w_attn[:, h0 : h0 + JW]),
                "wk": pack(w_attn[:, C + h0 : C + h0 + JW]),
                "bq": np.ascontiguousarray(
                    b_attn[h0 : h0 + JW].reshape(JW, 1).astype(np.float32)
                ),
            }
        in_maps.append(
            {
                **extra8,
                **wqk_entries,
                "xt": xt,
                "wv": pack(w_attn[:, 2 * C + h0 : 2 * C + h0 + JW]),
                "wp": _pack_wp(w_proj[h0 : h0 + JW, :]),
            }
        )
    return in_maps


_PROGRAM_CACHE = {}


def _get_program(B, T, C):
    key = (B, T, C)
    if key not in _PROGRAM_CACHE:
        _PROGRAM_CACHE[key] = build_program(B, T, C)
    return _PROGRAM_CACHE[key]


def kernel(x, w_attn, b_attn, w_proj, b_proj, _trace=False):
    B, T, C = x.shape
    nc = _get_program(B, T, C)
    in_maps = make_in_maps(x, w_attn, b_attn, w_proj)
    res = run_bass_kernel_spmd(nc, in_maps, list(range(N_CORES)), trace=_trace)
    out = np.zeros((B * T, C), np.float32)
    for r in res.results:
        part = np.asarray(r["out"], np.float32)
        if CFG.get("fo3_raw32", False):
            # last q-tile shipped as raw fp32 psum; its fp16 rows are unwritten
            part = part.copy()
            part[B * T - 128 :] = np.asarray(r["out32"], np.float32)
        out += part
    b_attn = np.asarray(b_attn, np.float32)
    w_proj = np.asarray(w_proj, np.float32)
    # v-bias commutes through softmax; k-bias is softmax-invariant
    out += (np.asarray(b_proj, np.float32) + b_attn[2 * C :] @ w_proj)[None, :]
    out = out.reshape(B, T, C)
    kernel.last_exec_time_ns = res.exec_time_ns
    return out

